# revision 35
# baseline (speedup 1.0000x reference)
"""Multi-head attention (S=2048, B=2, D=1024, H=16, Hd=64) on 8 trn2 cores.

Sharding: core = (batch b, head-group g of 4 heads)  -> 2*4 = 8 cores.
Each core computes the full attention for its 4 heads / 1 batch and a
partial output projection (row-parallel Wo); the host sums the 4 partials
per batch and adds bo.

Schedule (v3): software-pipelined around the ACT engine's exp wall.
  - 8 attention rounds of (sh in 4 s-blocks of 512, p in 2 head-pairs);
    per t-step the PE does 2 score mms (row-paired heads at tile_position
    0/64) + 2 attn chain mms (emitted with lag 2 so chain-buffer reuse
    stalls never block the score stream); ACT does one exp over
    [128, 1024] (both heads packed side by side in one PSUM score tile).
  - PSUM: scores 2x[128,1024] (4 banks) + chains 2x[128,512] (2 banks)
    + fill pool 2x[128,512] (2 banks) for proj/out-proj work that is
    interleaved into the rounds as PE filler (keeps the PE p-state up).
  - DMA order: wk, xk, wq, xq[sh0], wv, xv, xq[sh1..3], wo - so the
    k-projection starts as soon as the first xk tile lands and round 0
    starts right after q2[sh0]; the v-projection runs as round-0 filler
    (attn lag 4 there so it never blocks the score stream).
  - normalize: chains are drained to SBUF immediately (frees the chain
    PSUM bank for the next round after one DVE copy); Z goes partition
    64 -> 0 via a small gpsimd-issued SBUF DMA, then gpsimd
    partition_broadcast (which only honors partition-0 sources), DVE
    reciprocal and the scaling multiplies - all off the PE critical path.
  - out-proj is chunked per 128 output rows and interleaved as filler;
    each chunk DMAs out immediately from the SP queue.
"""

import sys

for _p in ("/opt/trn_rl_repo", "/root/.axon_site/_ro/trn_rl_repo"):
    if _p not in sys.path:
        sys.path.insert(0, _p)

import numpy as np
import ml_dtypes

S = 2048
B = 2
D = 1024
H = 16
HD = 64
NH = 4  # heads per core
P = 128
KD = D // P  # 8 contraction tiles for projections
NT = S // P  # 16 t tiles
WSC = 512  # s-columns per round
NSH = S // WSC  # 4 s-blocks

BF16 = ml_dtypes.bfloat16

_BUILD_CACHE = {}


def build_bass(s=S):
    """Build the per-core Bass module (same program for all 8 cores)."""
    import concourse.bacc as bacc
    import concourse.bass as bass
    import concourse.mybir as mybir
    import concourse.tile as tile

    f32 = mybir.dt.float32
    f32r = mybir.dt.float32r
    bf16 = mybir.dt.bfloat16
    AF = mybir.ActivationFunctionType
    ALU = mybir.AluOpType

    nc = bacc.Bacc("TRN2", target_bir_lowering=False, debug=False, num_devices=8)

    xq = nc.dram_tensor("xq_t", [D, s], bf16, kind="ExternalInput").ap()
    xk = nc.dram_tensor("xk_t", [D, s], bf16, kind="ExternalInput").ap()
    xv = nc.dram_tensor("xv_t", [D, s], bf16, kind="ExternalInput").ap()
    wq = nc.dram_tensor("wq_t", [D, 256], bf16, kind="ExternalInput").ap()
    wk = nc.dram_tensor("wk_t", [D, 256], bf16, kind="ExternalInput").ap()
    wv = nc.dram_tensor("wv_t", [D, 256], bf16, kind="ExternalInput").ap()
    wo = nc.dram_tensor("wo_h", [P, 2, D], bf16, kind="ExternalInput").ap()
    bq2 = nc.dram_tensor("bq2", [P, 2], f32, kind="ExternalInput").ap()
    bk2 = nc.dram_tensor("bk2", [P, 2], f32, kind="ExternalInput").ap()
    bv4 = nc.dram_tensor("bv4", [P, 256], f32, kind="ExternalInput").ap()
    out = nc.dram_tensor("out", [s, D], bf16, kind="ExternalOutput").ap()

    from contextlib import ExitStack

    with tile.TileContext(nc) as tc, ExitStack() as ctx:
        consts = ctx.enter_context(tc.tile_pool(name="consts", bufs=1))
        persist = ctx.enter_context(tc.tile_pool(name="persist", bufs=1))
        xkpool = ctx.enter_context(tc.tile_pool(name="xkpool", bufs=NSH))
        xvpool = ctx.enter_context(tc.tile_pool(name="xvpool", bufs=1))
        xqpool = ctx.enter_context(tc.tile_pool(name="xqpool", bufs=1))
        epool = ctx.enter_context(tc.tile_pool(name="epool", bufs=8))
        rzpool = ctx.enter_context(tc.tile_pool(name="rzpool", bufs=2))
        ospool = ctx.enter_context(tc.tile_pool(name="ospool", bufs=3))
        scp = ctx.enter_context(tc.tile_pool(name="scp", bufs=2, space="PSUM"))
        chp = ctx.enter_context(tc.tile_pool(name="chp", bufs=2, space="PSUM"))
        fillp = ctx.enter_context(tc.tile_pool(name="fillp", bufs=2, space="PSUM"))

        # ---- DMA order: wk, xk, wq, xq[sh0], wv, xv, xq[sh1..], wo ----
        wk_sb = consts.tile([P, KD, 256], bf16, name="wk_sb")
        nc.sync.dma_start(out=wk_sb, in_=wk.rearrange("(k p) e -> p k e", p=P))
        bk_sb = consts.tile([P, 2], f32, name="bk_sb")
        nc.sync.dma_start(out=bk_sb, in_=bk2)

        # few, large DMAs: each dma_start costs ~0.7us of serial SP issue
        # time. xk lands in four 512-column blocks so the first k-proj
        # chain (which contracts all k but only needs 512 s-columns)
        # starts as soon as block 0 arrives.
        xk3 = xk.rearrange("(k p) s -> p k s", p=P)
        xk_blocks = []
        for sh in range(NSH):
            t_ = xkpool.tile([P, KD, WSC], bf16, tag="xk", name=f"xk{sh}")
            nc.sync.dma_start(out=t_, in_=xk3[:, :, sh * WSC:(sh + 1) * WSC])
            xk_blocks.append(t_)

        wq_sb = consts.tile([P, KD, 256], bf16, name="wq_sb")
        nc.sync.dma_start(out=wq_sb, in_=wq.rearrange("(k p) e -> p k e", p=P))
        bq_sb = consts.tile([P, 2], f32, name="bq_sb")
        nc.sync.dma_start(out=bq_sb, in_=bq2)

        xq3 = xq.rearrange("(k p) s -> p k s", p=P)
        xq0_tile = xqpool.tile([P, KD, WSC], bf16, tag="xq0", name="xq0")
        nc.sync.dma_start(out=xq0_tile, in_=xq3[:, :, 0:WSC])

        wv_sb = consts.tile([P, KD, 256], bf16, name="wv_sb")
        nc.sync.dma_start(out=wv_sb, in_=wv.rearrange("(k p) e -> p k e", p=P))
        bv_sb = consts.tile([P, 256], f32, name="bv_sb")
        nc.sync.dma_start(out=bv_sb, in_=bv4)

        # xv and xq[sh1..3] ride the Activation engine's DMA queue - it
        # is idle during the lead-in and these loads have no waits, so
        # the transfers run in parallel with the SP-queue x loads
        xv_tile = xvpool.tile([P, KD, s], bf16, tag="xv", name="xv")
        nc.scalar.dma_start(out=xv_tile, in_=xv.rearrange("(k p) s -> p k s", p=P))

        xq1_tile = xqpool.tile([P, KD, NSH - 1, WSC], bf16, tag="xq1", name="xq1")
        nc.scalar.dma_start(out=xq1_tile, in_=xq3[:, :, WSC:])

        def xq_get(k, sh):
            return xq0_tile[:, k, :] if sh == 0 else xq1_tile[:, k, sh - 1, :]

        wo_sb = consts.tile([P, 2, D], bf16, name="wo_sb")
        nc.sync.dma_start(out=wo_sb, in_=wo)

        # ---- persistent activations -----------------------------------
        q2 = persist.tile([P, 2, s], bf16, name="q2")
        k2 = persist.tile([P, 2, s], bf16, name="k2")
        v_aug = persist.tile([P, NH, NT, 65], bf16, name="v_aug")
        nc.vector.memset(v_aug, 1.0)  # col 64 stays 1.0 = Z ones column
        ones_sb = consts.tile([1, 64], f32, name="ones_sb")
        nc.vector.memset(ones_sb, 1.0)  # lhsT for K=1 broadcast matmul
        # attn2: pair-packed normalized attention [128(e of 2 heads), 2, s]
        attn2 = persist.tile([P, 2, s], bf16, name="attn2")

        # ---- helpers (PE work runs in the fill PSUM pool) -------------
        def qk_proj(xget, w_sb, b_sb, dst, p, sh):
            # dst[:, p, sh-block] = (x @ W_pair.T)^T + bias  for 512 cols
            ps = fillp.tile([P, WSC], f32, tag="fill", name="qkps")
            for k in range(KD):
                nc.tensor.matmul(
                    ps,
                    lhsT=w_sb[:, k, p * P:(p + 1) * P],
                    rhs=xget(k, sh),
                    start=(k == 0),
                    stop=(k == KD - 1),
                )
            nc.vector.tensor_scalar(
                dst[:, p, sh * WSC:(sh + 1) * WSC], ps, b_sb[:, p:p + 1],
                None, ALU.add,
            )

        def v_proj(t):
            ps = fillp.tile([P, WSC], f32, tag="fill", name="vps")
            for k in range(KD):
                nc.tensor.matmul(
                    ps[:, 0:256],
                    lhsT=xv_tile[:, k, t * P:(t + 1) * P],
                    rhs=wv_sb[:, k, :],
                    start=(k == 0),
                    stop=(k == KD - 1),
                )
            for h in range(NH):
                nc.vector.tensor_tensor(
                    v_aug[:, h, t, 0:64],
                    ps[:, h * 64:(h + 1) * 64],
                    bv_sb[:, h * 64:(h + 1) * 64],
                    ALU.add,
                )

        def out_chunk(ci):
            # out rows [ci*128, (ci+1)*128) ; contract attn2 over both pairs
            ob = ospool.tile([P, D], bf16, tag="ob", name="ob")
            for nh_i in range(2):
                op = fillp.tile([P, WSC], f32, tag="fill", name="op")
                for p in range(2):
                    nc.tensor.matmul(
                        op,
                        lhsT=attn2[:, p, ci * P:(ci + 1) * P],
                        rhs=wo_sb[:, p, nh_i * 512:(nh_i + 1) * 512],
                        start=(p == 0),
                        stop=(p == 1),
                    )
                nc.vector.tensor_copy(ob[:, nh_i * 512:(nh_i + 1) * 512], op)
            nc.sync.dma_start(out=out[ci * P:(ci + 1) * P, :], in_=ob)

        def normalize(p, sh, ch0, ch1):
            soff = sh * WSC
            # drain chains to SBUF first: frees both chain banks after two
            # quick DVE copies so the next round's attn never waits long
            araw = rzpool.tile([P, 2, WSC], f32, tag="araw", name="araw")
            nc.vector.tensor_copy(araw[0:65, 0, :], ch0[0:65, :])
            nc.vector.tensor_copy(araw[0:65, 1, :], ch1[0:65, :])
            # Z (row 64): partition 64 -> 0 shift via gpsimd-issued DMA,
            # then broadcast (partition_broadcast needs a partition-0 src)
            z0 = rzpool.tile([1, 2, WSC], f32, tag="z0", name="z0")
            nc.sync.dma_start(out=z0, in_=araw[64:65])
            rz = rzpool.tile([64, 2, WSC], f32, tag="rz", name="rz")
            nc.gpsimd.partition_broadcast(rz, z0)
            nc.vector.reciprocal_approx_fast(rz, rz)
            # even head of pair -> attn2 rows 0:64 directly
            nc.vector.tensor_tensor(
                attn2[0:64, p, soff:soff + WSC],
                araw[0:64, 0, :],
                rz[:, 0, :],
                ALU.mult,
            )
            # odd head: scale to tmp then DMA-shift to rows 64:128
            atmp = rzpool.tile([HD, WSC], bf16, tag="atmp", name="atmp")
            nc.vector.tensor_tensor(atmp, araw[0:64, 1, :], rz[:, 1, :], ALU.mult)
            nc.sync.dma_start(
                out=attn2[64:128, p, soff:soff + WSC], in_=atmp
            )

        # ---- lead-in: k-proj (xk-block paced, sh-major), q(sh0) -------
        def xk_get(k, sh):
            return xk_blocks[sh][:, k, :]

        for sh in range(NSH):
            for p in range(2):
                qk_proj(xk_get, wk_sb, bk_sb, k2, p, sh)
        for p in range(2):
            qk_proj(xq_get, wq_sb, bq_sb, q2, p, 0)

        # ---- filler schedule ------------------------------------------
        # round r = sh*2 + p ; out-proj for sh needs rounds sh*2, sh*2+1
        # normalized, so its 4 chunks spread over rounds sh*2+2, sh*2+3.
        fillers = {r: {} for r in range(2 * NSH)}

        def add_filler(r, sl, job):
            fillers[r].setdefault(sl, []).append(job)

        # v-proj: round-0 filler; xv lands before round 0 starts, so pack
        # two per early slot (v(t) must land before attn(t) at slot t+2)
        for t in range(NT):
            add_filler(0, 2 + t // 2, lambda t=t: v_proj(t))
        qjobs = [(sh, p) for sh in range(1, NSH) for p in range(2)]
        qslots = [(1, 0), (1, 8), (2, 0), (3, 0), (4, 0), (4, 8)]
        for (r, sl), (sh, p) in zip(qslots, qjobs):
            add_filler(r, sl, lambda sh=sh, p=p: qk_proj(
                xq_get, wq_sb, bq_sb, q2, p, sh))
        # NOTE: out_chunk(sh) depends on normalize(sh*2+1), which is
        # emitted at slot 1 of round sh*2+2 - chunks there must sit at
        # slot >= 2 or the RAW dependency is never formed (stale read)
        oslots = {0: [(2, 4), (2, 12), (3, 4), (3, 12)],
                  1: [(4, 4), (4, 12), (5, 0), (5, 8)],
                  2: [(6, 2), (6, 9), (7, 0), (7, 8)]}
        for sh, slots in oslots.items():
            for j, (r, sl) in enumerate(slots):
                add_filler(r, sl, lambda ci=sh * 4 + j: out_chunk(ci))

        # ---- attention rounds -----------------------------------------
        # the previous round's attn-drain + normalize are emitted in the
        # first slots of the next round, so the score/exp stream never
        # waits behind them at a boundary
        pending = []
        for r in range(2 * NSH):
            sh, p = r // 2, r % 2
            soff = sh * WSC
            heads = (2 * p, 2 * p + 1)
            lag = 2
            ch0 = chp.tile([P, WSC], f32, tag="ch", name="ch0")
            ch1 = chp.tile([P, WSC], f32, tag="ch", name="ch1")
            ets = {}

            def attn_step(t, ch0=ch0, ch1=ch1, heads=heads, ets=ets):
                et = ets.pop(t)
                nc.tensor.matmul(
                    ch0[0:65, :],
                    lhsT=v_aug[:, heads[0], t, :],
                    rhs=et[:, 0:WSC],
                    start=(t == 0),
                    stop=(t == NT - 1),
                )
                nc.tensor.matmul(
                    ch1[0:65, :],
                    lhsT=v_aug[:, heads[1], t, :],
                    rhs=et[:, WSC:2 * WSC],
                    start=(t == 0),
                    stop=(t == NT - 1),
                )

            for t in range(NT):
                sc = scp.tile([P, 2 * WSC], f32, tag="sc", name="sc")
                for hi in range(2):
                    rlo, rhi = (0, 64) if hi == 0 else (64, 128)
                    nc.tensor.matmul(
                        sc[:, hi * WSC:(hi + 1) * WSC],
                        lhsT=k2[rlo:rhi, p, t * P:(t + 1) * P],
                        rhs=q2[rlo:rhi, p, soff:soff + WSC],
                        start=True,
                        stop=True,
                        tile_position=(rlo, 0),
                    )
                et = epool.tile([P, 2 * WSC], bf16, tag="exp", name="et")
                nc.scalar.activation(et, sc, AF.Exp, bias=0.0, scale=0.125)
                ets[t] = et
                if t == 0:  # drain previous round's chains
                    for job in pending[:-1]:
                        job()
                elif t == 1 and pending:
                    pending[-1]()  # previous round's normalize
                for job in fillers[r].get(t, []):
                    job()
                if t >= lag:
                    attn_step(t - lag)
            pending = [
                lambda t=t, f=attn_step: f(t) for t in range(NT - lag, NT)
            ]
            if r < 2 * NSH - 1:
                pending.append(
                    lambda p=p, sh=sh, a=ch0, b=ch1: normalize(p, sh, a, b)
                )
            last = (p, sh, ch0, ch1)

        # ---- tail: drain last round; sliced normalize + out-proj ------
        # (PE K=1 ones-matmul broadcast instead of the slow gpsimd
        # dispatch, 256-col slices so out-proj/DMA pipeline per slice)
        for job in pending:
            job()
        p, sh, ch0, ch1 = last
        soff = sh * WSC
        araw = rzpool.tile([P, 2, WSC], f32, tag="araw", name="araw_t")
        nc.vector.tensor_copy(araw[0:65, 0, :], ch0[0:65, :])
        nc.vector.tensor_copy(araw[0:65, 1, :], ch1[0:65, :])
        z0 = rzpool.tile([1, 2, WSC], f32, tag="z0", name="z0_t")
        nc.sync.dma_start(out=z0, in_=araw[64:65])
        zr = rzpool.tile([1, 2, WSC], f32, tag="zr", name="zr_t")
        nc.vector.reciprocal_approx_fast(zr, z0)
        HW_ = WSC // 2
        for sl in range(2):
            cs = slice(sl * HW_, (sl + 1) * HW_)
            ocs = slice(soff + sl * HW_, soff + (sl + 1) * HW_)
            rzp = fillp.tile([P, WSC], f32, tag="fill", name="rzp")
            nc.tensor.matmul(rzp[0:64, 0:HW_], lhsT=ones_sb,
                             rhs=zr[0:1, 0, cs], start=True, stop=True)
            nc.tensor.matmul(rzp[0:64, HW_:2 * HW_], lhsT=ones_sb,
                             rhs=zr[0:1, 1, cs], start=True, stop=True)
            nc.vector.tensor_tensor(
                attn2[0:64, p, ocs], araw[0:64, 0, cs],
                rzp[0:64, 0:HW_], ALU.mult,
            )
            atmp = rzpool.tile([HD, HW_], bf16, tag="atmp", name="atmp_t")
            nc.vector.tensor_tensor(
                atmp, araw[0:64, 1, cs], rzp[0:64, HW_:2 * HW_], ALU.mult,
            )
            nc.sync.dma_start(out=attn2[64:128, p, ocs], in_=atmp)
            out_chunk((NSH - 1) * 4 + 2 * sl)
            out_chunk((NSH - 1) * 4 + 2 * sl + 1)

    nc.compile()
    return nc


def get_bass(s=S):
    if s not in _BUILD_CACHE:
        _BUILD_CACHE[s] = build_bass(s)
    return _BUILD_CACHE[s]


def make_in_maps(query, key, value, Wq, bq, Wk, bk, Wv, bv, Wo):
    """Host-side sharding: per-core input dict for core = b*4 + g."""
    in_maps = []
    for core in range(8):
        b, g = core // 4, core % 4
        cs = slice(g * 256, (g + 1) * 256)
        # pair-packed: wo_h[hd + 64*(h%2), h//2, :] = Wo[:, g*256 + h*64 + hd]
        wo_h = (
            np.ascontiguousarray(Wo[:, cs].T)  # [256(h*64+hd), 1024]
            .reshape(2, P, D)
            .transpose(1, 0, 2)
        )
        m = {
            "xq_t": np.ascontiguousarray(query[:, b, :].T).astype(BF16),
            "xk_t": np.ascontiguousarray(key[:, b, :].T).astype(BF16),
            "xv_t": np.ascontiguousarray(value[:, b, :].T).astype(BF16),
            "wq_t": np.ascontiguousarray(Wq[cs, :].T).astype(BF16),
            "wk_t": np.ascontiguousarray(Wk[cs, :].T).astype(BF16),
            "wv_t": np.ascontiguousarray(Wv[cs, :].T).astype(BF16),
            "wo_h": np.ascontiguousarray(wo_h).astype(BF16),
            "bq2": np.ascontiguousarray(bq[cs].reshape(2, P).T).astype(np.float32),
            "bk2": np.ascontiguousarray(bk[cs].reshape(2, P).T).astype(np.float32),
            "bv4": np.ascontiguousarray(
                np.broadcast_to(bv[cs], (P, 256))
            ).astype(np.float32),
        }
        in_maps.append(m)
    return in_maps


def kernel(query, key, value, Wq, bq, Wk, bk, Wv, bv, Wo, bo):
    from concourse.bass_utils import run_bass_kernel_spmd

    query = np.asarray(query, dtype=np.float32)
    key = np.asarray(key, dtype=np.float32)
    value = np.asarray(value, dtype=np.float32)
    Wq = np.asarray(Wq, dtype=np.float32)
    Wk = np.asarray(Wk, dtype=np.float32)
    Wv = np.asarray(Wv, dtype=np.float32)
    Wo = np.asarray(Wo, dtype=np.float32)

    nc = get_bass(S)
    in_maps = make_in_maps(query, key, value, Wq, bq, Wk, bk, Wv, bv, Wo)
    res = run_bass_kernel_spmd(nc, in_maps, core_ids=list(range(8)))
    outs = [res.results[c]["out"] for c in range(8)]

    full = np.empty((S, B, D), dtype=np.float32)
    bo32 = np.asarray(bo, dtype=np.float32)
    for b in range(B):
        acc = outs[b * 4].astype(np.float32).copy()
        for g in range(1, 4):
            acc += outs[b * 4 + g]
        full[:, b, :] = acc + bo32[None, :]
    return full


# revision 37
# speedup vs baseline: 1.0435x; 1.0435x over previous
"""Multi-head attention (S=2048, B=2, D=1024, H=16, Hd=64) on 8 trn2 cores.

Sharding: core = (batch b, head-group g of 4 heads)  -> 2*4 = 8 cores.
Each core computes the full attention for its 4 heads / 1 batch and a
partial output projection (row-parallel Wo); the host sums the 4 partials
per batch and adds bo.

Schedule (v3): software-pipelined around the ACT engine's exp wall.
  - 8 attention rounds of (sh in 4 s-blocks of 512, p in 2 head-pairs);
    per t-step the PE does 2 score mms (row-paired heads at tile_position
    0/64) + 2 attn chain mms (emitted with lag 2 so chain-buffer reuse
    stalls never block the score stream); ACT does one exp over
    [128, 1024] (both heads packed side by side in one PSUM score tile).
  - PSUM: scores 2x[128,1024] (4 banks) + chains 2x[128,512] (2 banks)
    + fill pool 2x[128,512] (2 banks) for proj/out-proj work that is
    interleaved into the rounds as PE filler (keeps the PE p-state up).
  - DMA order: wk, xk, wq, xq[sh0], wv, xv, xq[sh1..3], wo - so the
    k-projection starts as soon as the first xk tile lands and round 0
    starts right after q2[sh0]; the v-projection runs as round-0 filler
    (attn lag 4 there so it never blocks the score stream).
  - normalize: chains are drained to SBUF immediately (frees the chain
    PSUM bank for the next round after one DVE copy); Z goes partition
    64 -> 0 via a small gpsimd-issued SBUF DMA, then gpsimd
    partition_broadcast (which only honors partition-0 sources), DVE
    reciprocal and the scaling multiplies - all off the PE critical path.
  - out-proj is chunked per 128 output rows and interleaved as filler;
    each chunk DMAs out immediately from the SP queue.
"""

import sys

for _p in ("/opt/trn_rl_repo", "/root/.axon_site/_ro/trn_rl_repo"):
    if _p not in sys.path:
        sys.path.insert(0, _p)

import numpy as np
import ml_dtypes

S = 2048
B = 2
D = 1024
H = 16
HD = 64
NH = 4  # heads per core
P = 128
KD = D // P  # 8 contraction tiles for projections
NT = S // P  # 16 t tiles
WSC = 512  # s-columns per round
NSH = S // WSC  # 4 s-blocks

BF16 = ml_dtypes.bfloat16

_BUILD_CACHE = {}


def build_bass(s=S):
    """Build the per-core Bass module (same program for all 8 cores)."""
    import concourse.bacc as bacc
    import concourse.bass as bass
    import concourse.mybir as mybir
    import concourse.tile as tile

    f32 = mybir.dt.float32
    f32r = mybir.dt.float32r
    bf16 = mybir.dt.bfloat16
    AF = mybir.ActivationFunctionType
    ALU = mybir.AluOpType

    nc = bacc.Bacc("TRN2", target_bir_lowering=False, debug=False, num_devices=8)

    xq = nc.dram_tensor("xq_t", [D, s], bf16, kind="ExternalInput").ap()
    xk = nc.dram_tensor("xk_t", [D, s], bf16, kind="ExternalInput").ap()
    xv = nc.dram_tensor("xv_t", [D, s], bf16, kind="ExternalInput").ap()
    wq = nc.dram_tensor("wq_t", [D, 256], bf16, kind="ExternalInput").ap()
    wk = nc.dram_tensor("wk_t", [D, 256], bf16, kind="ExternalInput").ap()
    wv = nc.dram_tensor("wv_t", [D, 256], bf16, kind="ExternalInput").ap()
    wo = nc.dram_tensor("wo_h", [P, 2, D], bf16, kind="ExternalInput").ap()
    bq2 = nc.dram_tensor("bq2", [P, 2], f32, kind="ExternalInput").ap()
    bk2 = nc.dram_tensor("bk2", [P, 2], f32, kind="ExternalInput").ap()
    bv4 = nc.dram_tensor("bv4", [P, 256], f32, kind="ExternalInput").ap()
    out = nc.dram_tensor("out", [s, D], bf16, kind="ExternalOutput").ap()

    from contextlib import ExitStack

    with tile.TileContext(nc) as tc, ExitStack() as ctx:
        consts = ctx.enter_context(tc.tile_pool(name="consts", bufs=1))
        persist = ctx.enter_context(tc.tile_pool(name="persist", bufs=1))
        xkpool = ctx.enter_context(tc.tile_pool(name="xkpool", bufs=NSH))
        xvpool = ctx.enter_context(tc.tile_pool(name="xvpool", bufs=1))
        xqpool = ctx.enter_context(tc.tile_pool(name="xqpool", bufs=1))
        epool = ctx.enter_context(tc.tile_pool(name="epool", bufs=8))
        rzpool = ctx.enter_context(tc.tile_pool(name="rzpool", bufs=2))
        ospool = ctx.enter_context(tc.tile_pool(name="ospool", bufs=3))
        scp = ctx.enter_context(tc.tile_pool(name="scp", bufs=2, space="PSUM"))
        chp = ctx.enter_context(tc.tile_pool(name="chp", bufs=2, space="PSUM"))
        fillp = ctx.enter_context(tc.tile_pool(name="fillp", bufs=2, space="PSUM"))

        # ---- DMA order: wk, xk, wq, xq[sh0], wv, xv, xq[sh1..], wo ----
        wk_sb = consts.tile([P, KD, 256], bf16, name="wk_sb")
        nc.sync.dma_start(out=wk_sb, in_=wk.rearrange("(k p) e -> p k e", p=P))
        bk_sb = consts.tile([P, 2], f32, name="bk_sb")
        nc.sync.dma_start(out=bk_sb, in_=bk2)

        # few, large DMAs: each dma_start costs ~0.7us of serial SP issue
        # time. xk lands in four 512-column blocks so the first k-proj
        # chain (which contracts all k but only needs 512 s-columns)
        # starts as soon as block 0 arrives.
        xk3 = xk.rearrange("(k p) s -> p k s", p=P)
        xk_blocks = []
        for sh in range(NSH):
            t_ = xkpool.tile([P, KD, WSC], bf16, tag="xk", name=f"xk{sh}")
            nc.sync.dma_start(out=t_, in_=xk3[:, :, sh * WSC:(sh + 1) * WSC])
            xk_blocks.append(t_)

        wq_sb = consts.tile([P, KD, 256], bf16, name="wq_sb")
        nc.sync.dma_start(out=wq_sb, in_=wq.rearrange("(k p) e -> p k e", p=P))
        bq_sb = consts.tile([P, 2], f32, name="bq_sb")
        nc.sync.dma_start(out=bq_sb, in_=bq2)

        xq3 = xq.rearrange("(k p) s -> p k s", p=P)
        xq0_tile = xqpool.tile([P, KD, WSC], bf16, tag="xq0", name="xq0")
        nc.sync.dma_start(out=xq0_tile, in_=xq3[:, :, 0:WSC])

        wv_sb = consts.tile([P, KD, 256], bf16, name="wv_sb")
        nc.sync.dma_start(out=wv_sb, in_=wv.rearrange("(k p) e -> p k e", p=P))
        bv_sb = consts.tile([P, 256], f32, name="bv_sb")
        nc.sync.dma_start(out=bv_sb, in_=bv4)

        xv_tile = xvpool.tile([P, KD, s], bf16, tag="xv", name="xv")
        nc.sync.dma_start(out=xv_tile, in_=xv.rearrange("(k p) s -> p k s", p=P))

        xq1_tile = xqpool.tile([P, KD, NSH - 1, WSC], bf16, tag="xq1", name="xq1")
        nc.sync.dma_start(out=xq1_tile, in_=xq3[:, :, WSC:])

        def xq_get(k, sh):
            return xq0_tile[:, k, :] if sh == 0 else xq1_tile[:, k, sh - 1, :]

        wo_sb = consts.tile([P, 2, D], bf16, name="wo_sb")
        nc.sync.dma_start(out=wo_sb, in_=wo)

        # ---- persistent activations -----------------------------------
        q2 = persist.tile([P, 2, s], bf16, name="q2")
        k2 = persist.tile([P, 2, s], bf16, name="k2")
        v_aug = persist.tile([P, NH, NT, 65], bf16, name="v_aug")
        nc.vector.memset(v_aug, 1.0)  # col 64 stays 1.0 = Z ones column
        ones_sb = consts.tile([1, 64], f32, name="ones_sb")
        nc.vector.memset(ones_sb, 1.0)  # lhsT for K=1 broadcast matmul
        # attn2: pair-packed normalized attention [128(e of 2 heads), 2, s]
        attn2 = persist.tile([P, 2, s], bf16, name="attn2")

        # ---- helpers (PE work runs in the fill PSUM pool) -------------
        def qk_proj(xget, w_sb, b_sb, dst, p, sh):
            # dst[:, p, sh-block] = (x @ W_pair.T)^T + bias  for 512 cols
            ps = fillp.tile([P, WSC], f32, tag="fill", name="qkps")
            for k in range(KD):
                nc.tensor.matmul(
                    ps,
                    lhsT=w_sb[:, k, p * P:(p + 1) * P],
                    rhs=xget(k, sh),
                    start=(k == 0),
                    stop=(k == KD - 1),
                )
            nc.vector.tensor_scalar(
                dst[:, p, sh * WSC:(sh + 1) * WSC], ps, b_sb[:, p:p + 1],
                None, ALU.add,
            )

        def v_proj(t):
            ps = fillp.tile([P, WSC], f32, tag="fill", name="vps")
            for k in range(KD):
                nc.tensor.matmul(
                    ps[:, 0:256],
                    lhsT=xv_tile[:, k, t * P:(t + 1) * P],
                    rhs=wv_sb[:, k, :],
                    start=(k == 0),
                    stop=(k == KD - 1),
                )
            for h in range(NH):
                nc.vector.tensor_tensor(
                    v_aug[:, h, t, 0:64],
                    ps[:, h * 64:(h + 1) * 64],
                    bv_sb[:, h * 64:(h + 1) * 64],
                    ALU.add,
                )

        def out_chunk(ci):
            # out rows [ci*128, (ci+1)*128) ; contract attn2 over both pairs
            ob = ospool.tile([P, D], bf16, tag="ob", name="ob")
            for nh_i in range(2):
                op = fillp.tile([P, WSC], f32, tag="fill", name="op")
                for p in range(2):
                    nc.tensor.matmul(
                        op,
                        lhsT=attn2[:, p, ci * P:(ci + 1) * P],
                        rhs=wo_sb[:, p, nh_i * 512:(nh_i + 1) * 512],
                        start=(p == 0),
                        stop=(p == 1),
                    )
                nc.vector.tensor_copy(ob[:, nh_i * 512:(nh_i + 1) * 512], op)
            nc.sync.dma_start(out=out[ci * P:(ci + 1) * P, :], in_=ob)

        def normalize(p, sh, ch0, ch1):
            soff = sh * WSC
            # drain chains to SBUF first: frees both chain banks after two
            # quick DVE copies so the next round's attn never waits long
            araw = rzpool.tile([P, 2, WSC], f32, tag="araw", name="araw")
            nc.vector.tensor_copy(araw[0:65, 0, :], ch0[0:65, :])
            nc.vector.tensor_copy(araw[0:65, 1, :], ch1[0:65, :])
            # Z (row 64): partition 64 -> 0 shift via gpsimd-issued DMA,
            # then broadcast (partition_broadcast needs a partition-0 src)
            z0 = rzpool.tile([1, 2, WSC], f32, tag="z0", name="z0")
            nc.sync.dma_start(out=z0, in_=araw[64:65])
            rz = rzpool.tile([64, 2, WSC], f32, tag="rz", name="rz")
            nc.gpsimd.partition_broadcast(rz, z0)
            nc.vector.reciprocal_approx_fast(rz, rz)
            # even head of pair -> attn2 rows 0:64 directly
            nc.vector.tensor_tensor(
                attn2[0:64, p, soff:soff + WSC],
                araw[0:64, 0, :],
                rz[:, 0, :],
                ALU.mult,
            )
            # odd head: scale to tmp then DMA-shift to rows 64:128
            atmp = rzpool.tile([HD, WSC], bf16, tag="atmp", name="atmp")
            nc.vector.tensor_tensor(atmp, araw[0:64, 1, :], rz[:, 1, :], ALU.mult)
            nc.sync.dma_start(
                out=attn2[64:128, p, soff:soff + WSC], in_=atmp
            )

        # ---- lead-in: k-proj (xk-block paced, sh-major), q(sh0) -------
        def xk_get(k, sh):
            return xk_blocks[sh][:, k, :]

        for sh in range(NSH):
            for p in range(2):
                qk_proj(xk_get, wk_sb, bk_sb, k2, p, sh)
        for p in range(2):
            qk_proj(xq_get, wq_sb, bq_sb, q2, p, 0)

        # ---- filler schedule ------------------------------------------
        # round r = sh*2 + p ; out-proj for sh needs rounds sh*2, sh*2+1
        # normalized, so its 4 chunks spread over rounds sh*2+2, sh*2+3.
        fillers = {r: {} for r in range(2 * NSH)}

        def add_filler(r, sl, job):
            fillers[r].setdefault(sl, []).append(job)

        # v-proj: round-0 filler; xv lands before round 0 starts, so pack
        # two per early slot (v(t) must land before attn(t) at slot t+2)
        for t in range(NT):
            add_filler(0, 2 + t // 2, lambda t=t: v_proj(t))
        qjobs = [(sh, p) for sh in range(1, NSH) for p in range(2)]
        qslots = [(1, 0), (1, 8), (2, 0), (3, 0), (4, 0), (4, 8)]
        for (r, sl), (sh, p) in zip(qslots, qjobs):
            add_filler(r, sl, lambda sh=sh, p=p: qk_proj(
                xq_get, wq_sb, bq_sb, q2, p, sh))
        # NOTE: out_chunk(sh) depends on normalize(sh*2+1), which is
        # emitted at slot 1 of round sh*2+2 - chunks there must sit at
        # slot >= 2 or the RAW dependency is never formed (stale read)
        oslots = {0: [(2, 4), (2, 12), (3, 4), (3, 12)],
                  1: [(4, 4), (4, 12), (5, 0), (5, 8)],
                  2: [(6, 2), (6, 9), (7, 0), (7, 8)]}
        for sh, slots in oslots.items():
            for j, (r, sl) in enumerate(slots):
                add_filler(r, sl, lambda ci=sh * 4 + j: out_chunk(ci))

        # ---- attention rounds -----------------------------------------
        # the previous round's attn-drain + normalize are emitted in the
        # first slots of the next round, so the score/exp stream never
        # waits behind them at a boundary
        pending = []
        for r in range(2 * NSH):
            sh, p = r // 2, r % 2
            soff = sh * WSC
            heads = (2 * p, 2 * p + 1)
            lag = 2
            ch0 = chp.tile([P, WSC], f32, tag="ch", name="ch0")
            ch1 = chp.tile([P, WSC], f32, tag="ch", name="ch1")
            ets = {}

            def attn_step(t, ch0=ch0, ch1=ch1, heads=heads, ets=ets):
                et = ets.pop(t)
                nc.tensor.matmul(
                    ch0[0:65, :],
                    lhsT=v_aug[:, heads[0], t, :],
                    rhs=et[:, 0:WSC],
                    start=(t == 0),
                    stop=(t == NT - 1),
                )
                nc.tensor.matmul(
                    ch1[0:65, :],
                    lhsT=v_aug[:, heads[1], t, :],
                    rhs=et[:, WSC:2 * WSC],
                    start=(t == 0),
                    stop=(t == NT - 1),
                )

            for t in range(NT):
                sc = scp.tile([P, 2 * WSC], f32, tag="sc", name="sc")
                for hi in range(2):
                    rlo, rhi = (0, 64) if hi == 0 else (64, 128)
                    nc.tensor.matmul(
                        sc[:, hi * WSC:(hi + 1) * WSC],
                        lhsT=k2[rlo:rhi, p, t * P:(t + 1) * P],
                        rhs=q2[rlo:rhi, p, soff:soff + WSC],
                        start=True,
                        stop=True,
                        tile_position=(rlo, 0),
                    )
                et = epool.tile([P, 2 * WSC], bf16, tag="exp", name="et")
                nc.scalar.activation(et, sc, AF.Exp, bias=0.0, scale=0.125)
                ets[t] = et
                if t == 0:  # drain previous round's chains
                    for job in pending[:-1]:
                        job()
                elif t == 1 and pending:
                    pending[-1]()  # previous round's normalize
                for job in fillers[r].get(t, []):
                    job()
                if t >= lag:
                    attn_step(t - lag)
            pending = [
                lambda t=t, f=attn_step: f(t) for t in range(NT - lag, NT)
            ]
            if r < 2 * NSH - 1:
                pending.append(
                    lambda p=p, sh=sh, a=ch0, b=ch1: normalize(p, sh, a, b)
                )
            last = (p, sh, ch0, ch1)

        # ---- tail: drain last round; sliced normalize + out-proj ------
        # (PE K=1 ones-matmul broadcast instead of the slow gpsimd
        # dispatch, 256-col slices so out-proj/DMA pipeline per slice)
        for job in pending:
            job()
        p, sh, ch0, ch1 = last
        soff = sh * WSC
        araw = rzpool.tile([P, 2, WSC], f32, tag="araw", name="araw_t")
        nc.vector.tensor_copy(araw[0:65, 0, :], ch0[0:65, :])
        nc.vector.tensor_copy(araw[0:65, 1, :], ch1[0:65, :])
        z0 = rzpool.tile([1, 2, WSC], f32, tag="z0", name="z0_t")
        nc.sync.dma_start(out=z0, in_=araw[64:65])
        zr = rzpool.tile([1, 2, WSC], f32, tag="zr", name="zr_t")
        nc.vector.reciprocal_approx_fast(zr, z0)
        HW_ = WSC // 2
        for sl in range(2):
            cs = slice(sl * HW_, (sl + 1) * HW_)
            ocs = slice(soff + sl * HW_, soff + (sl + 1) * HW_)
            rzp = fillp.tile([P, WSC], f32, tag="fill", name="rzp")
            nc.tensor.matmul(rzp[0:64, 0:HW_], lhsT=ones_sb,
                             rhs=zr[0:1, 0, cs], start=True, stop=True)
            nc.tensor.matmul(rzp[0:64, HW_:2 * HW_], lhsT=ones_sb,
                             rhs=zr[0:1, 1, cs], start=True, stop=True)
            nc.vector.tensor_tensor(
                attn2[0:64, p, ocs], araw[0:64, 0, cs],
                rzp[0:64, 0:HW_], ALU.mult,
            )
            atmp = rzpool.tile([HD, HW_], bf16, tag="atmp", name="atmp_t")
            nc.vector.tensor_tensor(
                atmp, araw[0:64, 1, cs], rzp[0:64, HW_:2 * HW_], ALU.mult,
            )
            nc.sync.dma_start(out=attn2[64:128, p, ocs], in_=atmp)
            out_chunk((NSH - 1) * 4 + 2 * sl)
            out_chunk((NSH - 1) * 4 + 2 * sl + 1)

    nc.compile()
    return nc


def get_bass(s=S):
    if s not in _BUILD_CACHE:
        _BUILD_CACHE[s] = build_bass(s)
    return _BUILD_CACHE[s]


def make_in_maps(query, key, value, Wq, bq, Wk, bk, Wv, bv, Wo):
    """Host-side sharding: per-core input dict for core = b*4 + g."""
    in_maps = []
    for core in range(8):
        b, g = core // 4, core % 4
        cs = slice(g * 256, (g + 1) * 256)
        # pair-packed: wo_h[hd + 64*(h%2), h//2, :] = Wo[:, g*256 + h*64 + hd]
        wo_h = (
            np.ascontiguousarray(Wo[:, cs].T)  # [256(h*64+hd), 1024]
            .reshape(2, P, D)
            .transpose(1, 0, 2)
        )
        m = {
            "xq_t": np.ascontiguousarray(query[:, b, :].T).astype(BF16),
            "xk_t": np.ascontiguousarray(key[:, b, :].T).astype(BF16),
            "xv_t": np.ascontiguousarray(value[:, b, :].T).astype(BF16),
            "wq_t": np.ascontiguousarray(Wq[cs, :].T).astype(BF16),
            "wk_t": np.ascontiguousarray(Wk[cs, :].T).astype(BF16),
            "wv_t": np.ascontiguousarray(Wv[cs, :].T).astype(BF16),
            "wo_h": np.ascontiguousarray(wo_h).astype(BF16),
            "bq2": np.ascontiguousarray(bq[cs].reshape(2, P).T).astype(np.float32),
            "bk2": np.ascontiguousarray(bk[cs].reshape(2, P).T).astype(np.float32),
            "bv4": np.ascontiguousarray(
                np.broadcast_to(bv[cs], (P, 256))
            ).astype(np.float32),
        }
        in_maps.append(m)
    return in_maps


def kernel(query, key, value, Wq, bq, Wk, bk, Wv, bv, Wo, bo):
    from concourse.bass_utils import run_bass_kernel_spmd

    query = np.asarray(query, dtype=np.float32)
    key = np.asarray(key, dtype=np.float32)
    value = np.asarray(value, dtype=np.float32)
    Wq = np.asarray(Wq, dtype=np.float32)
    Wk = np.asarray(Wk, dtype=np.float32)
    Wv = np.asarray(Wv, dtype=np.float32)
    Wo = np.asarray(Wo, dtype=np.float32)

    nc = get_bass(S)
    in_maps = make_in_maps(query, key, value, Wq, bq, Wk, bk, Wv, bv, Wo)
    res = run_bass_kernel_spmd(nc, in_maps, core_ids=list(range(8)))
    outs = [res.results[c]["out"] for c in range(8)]

    full = np.empty((S, B, D), dtype=np.float32)
    bo32 = np.asarray(bo, dtype=np.float32)
    for b in range(B):
        acc = outs[b * 4].astype(np.float32).copy()
        for g in range(1, 4):
            acc += outs[b * 4 + g]
        full[:, b, :] = acc + bo32[None, :]
    return full


# revision 45
# speedup vs baseline: 1.0836x; 1.0384x over previous
"""Multi-head attention (S=2048, B=2, D=1024, H=16, Hd=64) on 8 trn2 cores.

Sharding: core = (batch b, head-group g of 4 heads)  -> 2*4 = 8 cores.
Each core computes the full attention for its 4 heads / 1 batch and a
partial output projection (row-parallel Wo); the host sums the 4 partials
per batch and adds bo.

Schedule (v3): software-pipelined around the ACT engine's exp wall.
  - 8 attention rounds of (sh in 4 s-blocks of 512, p in 2 head-pairs);
    per t-step the PE does 2 score mms (row-paired heads at tile_position
    0/64) + 2 attn chain mms (emitted with lag 2 so chain-buffer reuse
    stalls never block the score stream); ACT does one exp over
    [128, 1024] (both heads packed side by side in one PSUM score tile).
  - PSUM: scores 2x[128,1024] (4 banks) + chains 2x[128,512] (2 banks)
    + fill pool 2x[128,512] (2 banks) for proj/out-proj work that is
    interleaved into the rounds as PE filler (keeps the PE p-state up).
  - DMA order: wk, xk, wq, xq[sh0], wv, xv, xq[sh1..3], wo - so the
    k-projection starts as soon as the first xk tile lands and round 0
    starts right after q2[sh0]; the v-projection runs as round-0 filler
    (attn lag 4 there so it never blocks the score stream).
  - normalize: chains are drained to SBUF immediately (frees the chain
    PSUM bank for the next round after one DVE copy); Z goes partition
    64 -> 0 via a small gpsimd-issued SBUF DMA, then gpsimd
    partition_broadcast (which only honors partition-0 sources), DVE
    reciprocal and the scaling multiplies - all off the PE critical path.
  - out-proj is chunked per 128 output rows and interleaved as filler;
    each chunk DMAs out immediately from the SP queue.
"""

import sys

for _p in ("/opt/trn_rl_repo", "/root/.axon_site/_ro/trn_rl_repo"):
    if _p not in sys.path:
        sys.path.insert(0, _p)

import numpy as np
import ml_dtypes

S = 2048
B = 2
D = 1024
H = 16
HD = 64
NH = 4  # heads per core
P = 128
KD = D // P  # 8 contraction tiles for projections
NT = S // P  # 16 t tiles
WSC = 512  # s-columns per round
NSH = S // WSC  # 4 s-blocks

BF16 = ml_dtypes.bfloat16

_BUILD_CACHE = {}


def build_bass(s=S):
    """Build the per-core Bass module (same program for all 8 cores)."""
    import concourse.bacc as bacc
    import concourse.bass as bass
    import concourse.mybir as mybir
    import concourse.tile as tile

    f32 = mybir.dt.float32
    f32r = mybir.dt.float32r
    bf16 = mybir.dt.bfloat16
    AF = mybir.ActivationFunctionType
    ALU = mybir.AluOpType

    nc = bacc.Bacc("TRN2", target_bir_lowering=False, debug=False, num_devices=8)

    xq = nc.dram_tensor("xq_t", [D, s], bf16, kind="ExternalInput").ap()
    xk = nc.dram_tensor("xk_t", [D, s], bf16, kind="ExternalInput").ap()
    xv = nc.dram_tensor("xv_t", [D, s], bf16, kind="ExternalInput").ap()
    wq = nc.dram_tensor("wq_t", [D, 256], bf16, kind="ExternalInput").ap()
    wk = nc.dram_tensor("wk_t", [D, 256], bf16, kind="ExternalInput").ap()
    wv = nc.dram_tensor("wv_t", [D, 256], bf16, kind="ExternalInput").ap()
    wo = nc.dram_tensor("wo_h", [P, 2, D], bf16, kind="ExternalInput").ap()
    bq2 = nc.dram_tensor("bq2", [P, 2], f32, kind="ExternalInput").ap()
    bk2 = nc.dram_tensor("bk2", [P, 2], f32, kind="ExternalInput").ap()
    bv4 = nc.dram_tensor("bv4", [P, 256], f32, kind="ExternalInput").ap()
    out = nc.dram_tensor("out", [s, D], bf16, kind="ExternalOutput").ap()

    from contextlib import ExitStack

    with tile.TileContext(nc) as tc, ExitStack() as ctx:
        consts = ctx.enter_context(tc.tile_pool(name="consts", bufs=1))
        persist = ctx.enter_context(tc.tile_pool(name="persist", bufs=1))
        xkpool = ctx.enter_context(tc.tile_pool(name="xkpool", bufs=NSH))
        xvpool = ctx.enter_context(tc.tile_pool(name="xvpool", bufs=NSH))
        xqpool = ctx.enter_context(tc.tile_pool(name="xqpool", bufs=1))
        epool = ctx.enter_context(tc.tile_pool(name="epool", bufs=8))
        rzpool = ctx.enter_context(tc.tile_pool(name="rzpool", bufs=2))
        ospool = ctx.enter_context(tc.tile_pool(name="ospool", bufs=3))
        scp = ctx.enter_context(tc.tile_pool(name="scp", bufs=2, space="PSUM"))
        chp = ctx.enter_context(tc.tile_pool(name="chp", bufs=2, space="PSUM"))
        fillp = ctx.enter_context(tc.tile_pool(name="fillp", bufs=2, space="PSUM"))

        # ---- DMA order: wk, xk, wq, xq[sh0], wv, xv, xq[sh1..], wo ----
        wk_sb = consts.tile([P, KD, 256], bf16, name="wk_sb")
        nc.sync.dma_start(out=wk_sb, in_=wk.rearrange("(k p) e -> p k e", p=P))
        bk_sb = consts.tile([P, 2], f32, name="bk_sb")
        nc.sync.dma_start(out=bk_sb, in_=bk2)

        # few, large DMAs: each dma_start costs ~0.7us of serial SP issue
        # time. xk lands in four 512-column blocks so the first k-proj
        # chain (which contracts all k but only needs 512 s-columns)
        # starts as soon as block 0 arrives.
        xk3 = xk.rearrange("(k p) s -> p k s", p=P)
        xk_blocks = []
        for sh in range(NSH):
            t_ = xkpool.tile([P, KD, WSC], bf16, tag="xk", name=f"xk{sh}")
            nc.sync.dma_start(out=t_, in_=xk3[:, :, sh * WSC:(sh + 1) * WSC])
            xk_blocks.append(t_)

        wq_sb = consts.tile([P, KD, 256], bf16, name="wq_sb")
        nc.sync.dma_start(out=wq_sb, in_=wq.rearrange("(k p) e -> p k e", p=P))
        bq_sb = consts.tile([P, 2], f32, name="bq_sb")
        nc.sync.dma_start(out=bq_sb, in_=bq2)

        xq3 = xq.rearrange("(k p) s -> p k s", p=P)
        xq0_tile = xqpool.tile([P, KD, WSC], bf16, tag="xq0", name="xq0")
        nc.sync.dma_start(out=xq0_tile, in_=xq3[:, :, 0:WSC])

        wv_sb = consts.tile([P, KD, 256], bf16, name="wv_sb")
        nc.sync.dma_start(out=wv_sb, in_=wv.rearrange("(k p) e -> p k e", p=P))
        bv_sb = consts.tile([P, 256], f32, name="bv_sb")
        nc.sync.dma_start(out=bv_sb, in_=bv4)

        # xv in four t-column blocks: v_proj(t) only waits for its block
        xv4 = xv.rearrange("(k p) s -> p k s", p=P)
        xv_blocks = []
        for b in range(NSH):
            t_ = xvpool.tile([P, KD, WSC], bf16, tag="xv", name=f"xv{b}")
            nc.sync.dma_start(out=t_, in_=xv4[:, :, b * WSC:(b + 1) * WSC])
            xv_blocks.append(t_)

        xq1_tile = xqpool.tile([P, KD, NSH - 1, WSC], bf16, tag="xq1", name="xq1")
        nc.sync.dma_start(out=xq1_tile, in_=xq3[:, :, WSC:])

        def xq_get(k, sh):
            return xq0_tile[:, k, :] if sh == 0 else xq1_tile[:, k, sh - 1, :]

        wo_sb = consts.tile([P, 2, D], bf16, name="wo_sb")
        nc.sync.dma_start(out=wo_sb, in_=wo)

        # ---- persistent activations -----------------------------------
        q2 = persist.tile([P, 2, s], bf16, name="q2")
        k2 = persist.tile([P, 2, s], bf16, name="k2")
        v_aug = persist.tile([P, NH, NT, 65], bf16, name="v_aug")
        nc.vector.memset(v_aug, 1.0)  # col 64 stays 1.0 = Z ones column
        ones_sb = consts.tile([1, 64], f32, name="ones_sb")
        nc.vector.memset(ones_sb, 1.0)  # lhsT for K=1 broadcast matmul
        # attn2: pair-packed normalized attention [128(e of 2 heads), 2, s]
        attn2 = persist.tile([P, 2, s], bf16, name="attn2")

        # ---- helpers (PE work runs in the fill PSUM pool) -------------
        def qk_proj(xget, w_sb, b_sb, dst, p, sh):
            # dst[:, p, sh-block] = (x @ W_pair.T)^T + bias  for 512 cols
            ps = fillp.tile([P, WSC], f32, tag="fill", name="qkps")
            for k in range(KD):
                nc.tensor.matmul(
                    ps,
                    lhsT=w_sb[:, k, p * P:(p + 1) * P],
                    rhs=xget(k, sh),
                    start=(k == 0),
                    stop=(k == KD - 1),
                )
            nc.vector.tensor_scalar(
                dst[:, p, sh * WSC:(sh + 1) * WSC], ps, b_sb[:, p:p + 1],
                None, ALU.add,
            )

        def v_proj(t):
            ps = fillp.tile([P, WSC], f32, tag="fill", name="vps")
            xvb = xv_blocks[t // 4]
            toff = (t % 4) * P
            for k in range(KD):
                nc.tensor.matmul(
                    ps[:, 0:256],
                    lhsT=xvb[:, k, toff:toff + P],
                    rhs=wv_sb[:, k, :],
                    start=(k == 0),
                    stop=(k == KD - 1),
                )
            for h in range(NH):
                nc.vector.tensor_tensor(
                    v_aug[:, h, t, 0:64],
                    ps[:, h * 64:(h + 1) * 64],
                    bv_sb[:, h * 64:(h + 1) * 64],
                    ALU.add,
                )

        def out_chunk(ci):
            # out rows [ci*128, (ci+1)*128) ; contract attn2 over both pairs
            ob = ospool.tile([P, D], bf16, tag="ob", name="ob")
            for nh_i in range(2):
                op = fillp.tile([P, WSC], f32, tag="fill", name="op")
                for p in range(2):
                    nc.tensor.matmul(
                        op,
                        lhsT=attn2[:, p, ci * P:(ci + 1) * P],
                        rhs=wo_sb[:, p, nh_i * 512:(nh_i + 1) * 512],
                        start=(p == 0),
                        stop=(p == 1),
                    )
                nc.vector.tensor_copy(ob[:, nh_i * 512:(nh_i + 1) * 512], op)
            nc.sync.dma_start(out=out[ci * P:(ci + 1) * P, :], in_=ob)

        def normalize(p, sh, ch0, ch1):
            soff = sh * WSC
            # drain chains to SBUF first: frees both chain banks after two
            # quick DVE copies so the next round's attn never waits long
            araw = rzpool.tile([P, 2, WSC], f32, tag="araw", name="araw")
            nc.vector.tensor_copy(araw[0:65, 0, :], ch0[0:65, :])
            nc.vector.tensor_copy(araw[0:65, 1, :], ch1[0:65, :])
            # Z (row 64): partition 64 -> 0 shift via SBUF DMA, then
            # broadcast (partition_broadcast needs a partition-0 src)
            z0 = rzpool.tile([1, 2, WSC], f32, tag="z0", name="z0")
            nc.sync.dma_start(out=z0, in_=araw[64:65])
            rz = rzpool.tile([64, 2, WSC], f32, tag="rz", name="rz")
            nc.gpsimd.partition_broadcast(rz, z0)
            nc.vector.reciprocal_approx_fast(rz, rz)
            # even head of pair -> attn2 rows 0:64 directly
            nc.vector.tensor_tensor(
                attn2[0:64, p, soff:soff + WSC],
                araw[0:64, 0, :],
                rz[:, 0, :],
                ALU.mult,
            )
            # odd head: scale to tmp then DMA-shift to rows 64:128
            atmp = rzpool.tile([HD, WSC], bf16, tag="atmp", name="atmp")
            nc.vector.tensor_tensor(atmp, araw[0:64, 1, :], rz[:, 1, :], ALU.mult)
            nc.sync.dma_start(
                out=attn2[64:128, p, soff:soff + WSC], in_=atmp
            )

        # ---- lead-in: k-proj (xk-block paced, sh-major), q(sh0) -------
        def xk_get(k, sh):
            return xk_blocks[sh][:, k, :]

        for sh in range(NSH):
            for p in range(2):
                qk_proj(xk_get, wk_sb, bk_sb, k2, p, sh)
        for p in range(2):
            qk_proj(xq_get, wq_sb, bq_sb, q2, p, 0)

        # ---- filler schedule ------------------------------------------
        # round r = sh*2 + p ; out-proj for sh needs rounds sh*2, sh*2+1
        # normalized, so its 4 chunks spread over rounds sh*2+2, sh*2+3.
        fillers = {r: {} for r in range(2 * NSH)}

        def add_filler(r, sl, job):
            fillers[r].setdefault(sl, []).append(job)

        # v-proj: round-0 filler; xv lands before round 0 starts, so pack
        # two per early slot (v(t) must land before attn(t) at slot t+2)
        for t in range(NT):
            add_filler(0, 2 + (t * 13) // 16, lambda t=t: v_proj(t))
        qjobs = [(sh, p) for sh in range(1, NSH) for p in range(2)]
        qslots = [(1, 0), (1, 8), (2, 0), (3, 0), (4, 0), (4, 8)]
        for (r, sl), (sh, p) in zip(qslots, qjobs):
            add_filler(r, sl, lambda sh=sh, p=p: qk_proj(
                xq_get, wq_sb, bq_sb, q2, p, sh))
        # NOTE: out_chunk(sh) depends on normalize(sh*2+1), which is
        # emitted at slot 1 of round sh*2+2 - chunks there must sit at
        # slot >= 2 or the RAW dependency is never formed (stale read)
        oslots = {0: [(2, 4), (2, 12), (3, 4), (3, 12)],
                  1: [(4, 4), (4, 12), (5, 0), (5, 8)],
                  2: [(6, 2), (6, 9), (7, 0), (7, 8)]}
        for sh, slots in oslots.items():
            for j, (r, sl) in enumerate(slots):
                add_filler(r, sl, lambda ci=sh * 4 + j: out_chunk(ci))

        # ---- attention rounds -----------------------------------------
        # the previous round's attn-drain + normalize are emitted in the
        # first slots of the next round, so the score/exp stream never
        # waits behind them at a boundary
        pending = []
        for r in range(2 * NSH):
            sh, p = r // 2, r % 2
            soff = sh * WSC
            heads = (2 * p, 2 * p + 1)
            lag = 2
            ch0 = chp.tile([P, WSC], f32, tag="ch", name="ch0")
            ch1 = chp.tile([P, WSC], f32, tag="ch", name="ch1")
            ets = {}

            def attn_step(t, ch0=ch0, ch1=ch1, heads=heads, ets=ets):
                et = ets.pop(t)
                nc.tensor.matmul(
                    ch0[0:65, :],
                    lhsT=v_aug[:, heads[0], t, :],
                    rhs=et[:, 0:WSC],
                    start=(t == 0),
                    stop=(t == NT - 1),
                )
                nc.tensor.matmul(
                    ch1[0:65, :],
                    lhsT=v_aug[:, heads[1], t, :],
                    rhs=et[:, WSC:2 * WSC],
                    start=(t == 0),
                    stop=(t == NT - 1),
                )

            for t in range(NT):
                sc = scp.tile([P, 2 * WSC], f32, tag="sc", name="sc")
                for hi in range(2):
                    rlo, rhi = (0, 64) if hi == 0 else (64, 128)
                    nc.tensor.matmul(
                        sc[:, hi * WSC:(hi + 1) * WSC],
                        lhsT=k2[rlo:rhi, p, t * P:(t + 1) * P],
                        rhs=q2[rlo:rhi, p, soff:soff + WSC],
                        start=True,
                        stop=True,
                        tile_position=(rlo, 0),
                    )
                et = epool.tile([P, 2 * WSC], bf16, tag="exp", name="et")
                nc.scalar.activation(et, sc, AF.Exp, bias=0.0, scale=0.125)
                ets[t] = et
                if t == 0:  # drain previous round's chains
                    for job in pending[:-1]:
                        job()
                elif t == 1 and pending:
                    pending[-1]()  # previous round's normalize
                for job in fillers[r].get(t, []):
                    job()
                if t >= lag:
                    attn_step(t - lag)
            pending = [
                lambda t=t, f=attn_step: f(t) for t in range(NT - lag, NT)
            ]
            if r < 2 * NSH - 1:
                pending.append(
                    lambda p=p, sh=sh, a=ch0, b=ch1: normalize(p, sh, a, b)
                )
            last = (p, sh, ch0, ch1)

        # ---- tail: drain last round; sliced normalize + out-proj ------
        # (PE K=1 ones-matmul broadcast instead of the slow gpsimd
        # dispatch, 256-col slices so out-proj/DMA pipeline per slice)
        for job in pending:
            job()
        p, sh, ch0, ch1 = last
        soff = sh * WSC
        araw = rzpool.tile([P, 2, WSC], f32, tag="araw", name="araw_t")
        nc.vector.tensor_copy(araw[0:65, 0, :], ch0[0:65, :])
        nc.vector.tensor_copy(araw[0:65, 1, :], ch1[0:65, :])
        z0 = rzpool.tile([1, 2, WSC], f32, tag="z0", name="z0_t")
        nc.sync.dma_start(out=z0, in_=araw[64:65])
        zr = rzpool.tile([1, 2, WSC], f32, tag="zr", name="zr_t")
        nc.vector.reciprocal_approx_fast(zr, z0)
        HW_ = WSC // 2
        for sl in range(2):
            cs = slice(sl * HW_, (sl + 1) * HW_)
            ocs = slice(soff + sl * HW_, soff + (sl + 1) * HW_)
            rzp = fillp.tile([P, WSC], f32, tag="fill", name="rzp")
            nc.tensor.matmul(rzp[0:64, 0:HW_], lhsT=ones_sb,
                             rhs=zr[0:1, 0, cs], start=True, stop=True)
            nc.tensor.matmul(rzp[0:64, HW_:2 * HW_], lhsT=ones_sb,
                             rhs=zr[0:1, 1, cs], start=True, stop=True)
            nc.vector.tensor_tensor(
                attn2[0:64, p, ocs], araw[0:64, 0, cs],
                rzp[0:64, 0:HW_], ALU.mult,
            )
            atmp = rzpool.tile([HD, HW_], bf16, tag="atmp", name="atmp_t")
            nc.vector.tensor_tensor(
                atmp, araw[0:64, 1, cs], rzp[0:64, HW_:2 * HW_], ALU.mult,
            )
            nc.sync.dma_start(out=attn2[64:128, p, ocs], in_=atmp)
            out_chunk((NSH - 1) * 4 + 2 * sl)
            out_chunk((NSH - 1) * 4 + 2 * sl + 1)

    nc.compile()
    return nc


def get_bass(s=S):
    if s not in _BUILD_CACHE:
        _BUILD_CACHE[s] = build_bass(s)
    return _BUILD_CACHE[s]


def make_in_maps(query, key, value, Wq, bq, Wk, bk, Wv, bv, Wo):
    """Host-side sharding: per-core input dict for core = b*4 + g."""
    in_maps = []
    for core in range(8):
        b, g = core // 4, core % 4
        cs = slice(g * 256, (g + 1) * 256)
        # pair-packed: wo_h[hd + 64*(h%2), h//2, :] = Wo[:, g*256 + h*64 + hd]
        wo_h = (
            np.ascontiguousarray(Wo[:, cs].T)  # [256(h*64+hd), 1024]
            .reshape(2, P, D)
            .transpose(1, 0, 2)
        )
        m = {
            "xq_t": np.ascontiguousarray(query[:, b, :].T).astype(BF16),
            "xk_t": np.ascontiguousarray(key[:, b, :].T).astype(BF16),
            "xv_t": np.ascontiguousarray(value[:, b, :].T).astype(BF16),
            "wq_t": np.ascontiguousarray(Wq[cs, :].T).astype(BF16),
            "wk_t": np.ascontiguousarray(Wk[cs, :].T).astype(BF16),
            "wv_t": np.ascontiguousarray(Wv[cs, :].T).astype(BF16),
            "wo_h": np.ascontiguousarray(wo_h).astype(BF16),
            "bq2": np.ascontiguousarray(bq[cs].reshape(2, P).T).astype(np.float32),
            "bk2": np.ascontiguousarray(bk[cs].reshape(2, P).T).astype(np.float32),
            "bv4": np.ascontiguousarray(
                np.broadcast_to(bv[cs], (P, 256))
            ).astype(np.float32),
        }
        in_maps.append(m)
    return in_maps


def kernel(query, key, value, Wq, bq, Wk, bk, Wv, bv, Wo, bo):
    from concourse.bass_utils import run_bass_kernel_spmd

    query = np.asarray(query, dtype=np.float32)
    key = np.asarray(key, dtype=np.float32)
    value = np.asarray(value, dtype=np.float32)
    Wq = np.asarray(Wq, dtype=np.float32)
    Wk = np.asarray(Wk, dtype=np.float32)
    Wv = np.asarray(Wv, dtype=np.float32)
    Wo = np.asarray(Wo, dtype=np.float32)

    nc = get_bass(S)
    in_maps = make_in_maps(query, key, value, Wq, bq, Wk, bk, Wv, bv, Wo)
    res = run_bass_kernel_spmd(nc, in_maps, core_ids=list(range(8)))
    outs = [res.results[c]["out"] for c in range(8)]

    full = np.empty((S, B, D), dtype=np.float32)
    bo32 = np.asarray(bo, dtype=np.float32)
    for b in range(B):
        acc = outs[b * 4].astype(np.float32).copy()
        for g in range(1, 4):
            acc += outs[b * 4 + g]
        full[:, b, :] = acc + bo32[None, :]
    return full


# revision 47
# speedup vs baseline: 1.0941x; 1.0097x over previous
"""Multi-head attention (S=2048, B=2, D=1024, H=16, Hd=64) on 8 trn2 cores.

Sharding: core = (batch b, head-group g of 4 heads)  -> 2*4 = 8 cores.
Each core computes the full attention for its 4 heads / 1 batch and a
partial output projection (row-parallel Wo); the host sums the 4 partials
per batch and adds bo.

Schedule (v3): software-pipelined around the ACT engine's exp wall.
  - 8 attention rounds of (sh in 4 s-blocks of 512, p in 2 head-pairs);
    per t-step the PE does 2 score mms (row-paired heads at tile_position
    0/64) + 2 attn chain mms (emitted with lag 2 so chain-buffer reuse
    stalls never block the score stream); ACT does one exp over
    [128, 1024] (both heads packed side by side in one PSUM score tile).
  - PSUM: scores 2x[128,1024] (4 banks) + chains 2x[128,512] (2 banks)
    + fill pool 2x[128,512] (2 banks) for proj/out-proj work that is
    interleaved into the rounds as PE filler (keeps the PE p-state up).
  - DMA order: wk, xk, wq, xq[sh0], wv, xv, xq[sh1..3], wo - so the
    k-projection starts as soon as the first xk tile lands and round 0
    starts right after q2[sh0]; the v-projection runs as round-0 filler
    (attn lag 4 there so it never blocks the score stream).
  - normalize: chains are drained to SBUF immediately (frees the chain
    PSUM bank for the next round after one DVE copy); Z goes partition
    64 -> 0 via a small gpsimd-issued SBUF DMA, then gpsimd
    partition_broadcast (which only honors partition-0 sources), DVE
    reciprocal and the scaling multiplies - all off the PE critical path.
  - out-proj is chunked per 128 output rows and interleaved as filler;
    each chunk DMAs out immediately from the SP queue.
"""

import sys

for _p in ("/opt/trn_rl_repo", "/root/.axon_site/_ro/trn_rl_repo"):
    if _p not in sys.path:
        sys.path.insert(0, _p)

import numpy as np
import ml_dtypes

S = 2048
B = 2
D = 1024
H = 16
HD = 64
NH = 4  # heads per core
P = 128
KD = D // P  # 8 contraction tiles for projections
NT = S // P  # 16 t tiles
WSC = 512  # s-columns per round
NSH = S // WSC  # 4 s-blocks

BF16 = ml_dtypes.bfloat16

_BUILD_CACHE = {}


def build_bass(s=S):
    """Build the per-core Bass module (same program for all 8 cores)."""
    import concourse.bacc as bacc
    import concourse.bass as bass
    import concourse.mybir as mybir
    import concourse.tile as tile

    f32 = mybir.dt.float32
    f32r = mybir.dt.float32r
    bf16 = mybir.dt.bfloat16
    AF = mybir.ActivationFunctionType
    ALU = mybir.AluOpType

    nc = bacc.Bacc("TRN2", target_bir_lowering=False, debug=False, num_devices=8)

    xq = nc.dram_tensor("xq_t", [D, s], bf16, kind="ExternalInput").ap()
    xk = nc.dram_tensor("xk_t", [D, s], bf16, kind="ExternalInput").ap()
    xv = nc.dram_tensor("xv_t", [D, s], bf16, kind="ExternalInput").ap()
    wq = nc.dram_tensor("wq_t", [D, 256], bf16, kind="ExternalInput").ap()
    wk = nc.dram_tensor("wk_t", [D, 256], bf16, kind="ExternalInput").ap()
    wv = nc.dram_tensor("wv_t", [D, 256], bf16, kind="ExternalInput").ap()
    wo = nc.dram_tensor("wo_h", [P, 2, D], bf16, kind="ExternalInput").ap()
    bq2 = nc.dram_tensor("bq2", [P, 2], f32, kind="ExternalInput").ap()
    bk2 = nc.dram_tensor("bk2", [P, 2], f32, kind="ExternalInput").ap()
    bv4 = nc.dram_tensor("bv4", [P, 256], f32, kind="ExternalInput").ap()
    out = nc.dram_tensor("out", [s, D], bf16, kind="ExternalOutput").ap()

    from contextlib import ExitStack

    with tile.TileContext(nc) as tc, ExitStack() as ctx:
        consts = ctx.enter_context(tc.tile_pool(name="consts", bufs=1))
        persist = ctx.enter_context(tc.tile_pool(name="persist", bufs=1))
        xkpool = ctx.enter_context(tc.tile_pool(name="xkpool", bufs=NSH))
        xvpool = ctx.enter_context(tc.tile_pool(name="xvpool", bufs=NSH))
        xqpool = ctx.enter_context(tc.tile_pool(name="xqpool", bufs=1))
        epool = ctx.enter_context(tc.tile_pool(name="epool", bufs=8))
        rzpool = ctx.enter_context(tc.tile_pool(name="rzpool", bufs=2))
        ospool = ctx.enter_context(tc.tile_pool(name="ospool", bufs=3))
        scp = ctx.enter_context(tc.tile_pool(name="scp", bufs=2, space="PSUM"))
        chp = ctx.enter_context(tc.tile_pool(name="chp", bufs=2, space="PSUM"))
        fillp = ctx.enter_context(tc.tile_pool(name="fillp", bufs=2, space="PSUM"))

        # ---- DMA order: wk, xk, wq, xq[sh0], wv, xv, xq[sh1..], wo ----
        wk_sb = consts.tile([P, KD, 256], bf16, name="wk_sb")
        nc.sync.dma_start(out=wk_sb, in_=wk.rearrange("(k p) e -> p k e", p=P))
        bk_sb = consts.tile([P, 2], f32, name="bk_sb")
        nc.sync.dma_start(out=bk_sb, in_=bk2)

        # few, large DMAs: each dma_start costs ~0.7us of serial SP issue
        # time. xk lands in four 512-column blocks so the first k-proj
        # chain (which contracts all k but only needs 512 s-columns)
        # starts as soon as block 0 arrives.
        xk3 = xk.rearrange("(k p) s -> p k s", p=P)
        xk_blocks = []
        for sh in range(NSH):
            t_ = xkpool.tile([P, KD, WSC], bf16, tag="xk", name=f"xk{sh}")
            nc.sync.dma_start(out=t_, in_=xk3[:, :, sh * WSC:(sh + 1) * WSC])
            xk_blocks.append(t_)

        wq_sb = consts.tile([P, KD, 256], bf16, name="wq_sb")
        nc.sync.dma_start(out=wq_sb, in_=wq.rearrange("(k p) e -> p k e", p=P))
        bq_sb = consts.tile([P, 2], f32, name="bq_sb")
        nc.sync.dma_start(out=bq_sb, in_=bq2)

        xq3 = xq.rearrange("(k p) s -> p k s", p=P)
        xq0_tile = xqpool.tile([P, KD, WSC], bf16, tag="xq0", name="xq0")
        nc.sync.dma_start(out=xq0_tile, in_=xq3[:, :, 0:WSC])

        wv_sb = consts.tile([P, KD, 256], bf16, name="wv_sb")
        nc.sync.dma_start(out=wv_sb, in_=wv.rearrange("(k p) e -> p k e", p=P))
        bv_sb = consts.tile([P, 256], f32, name="bv_sb")
        nc.sync.dma_start(out=bv_sb, in_=bv4)

        # xv in four t-column blocks: v_proj(t) only waits for its block
        xv4 = xv.rearrange("(k p) s -> p k s", p=P)
        xv_blocks = []
        for b in range(NSH):
            t_ = xvpool.tile([P, KD, WSC], bf16, tag="xv", name=f"xv{b}")
            nc.sync.dma_start(out=t_, in_=xv4[:, :, b * WSC:(b + 1) * WSC])
            xv_blocks.append(t_)

        xq1_tile = xqpool.tile([P, KD, NSH - 1, WSC], bf16, tag="xq1", name="xq1")
        nc.sync.dma_start(out=xq1_tile, in_=xq3[:, :, WSC:])

        def xq_get(k, sh):
            return xq0_tile[:, k, :] if sh == 0 else xq1_tile[:, k, sh - 1, :]

        wo_sb = consts.tile([P, 2, D], bf16, name="wo_sb")
        nc.sync.dma_start(out=wo_sb, in_=wo)

        # ---- persistent activations -----------------------------------
        q2 = persist.tile([P, 2, s], bf16, name="q2")
        k2 = persist.tile([P, 2, s], bf16, name="k2")
        v_aug = persist.tile([P, NH, NT, 65], bf16, name="v_aug")
        nc.vector.memset(v_aug, 1.0)  # col 64 stays 1.0 = Z ones column
        ones_sb = consts.tile([1, 64], f32, name="ones_sb")
        nc.vector.memset(ones_sb, 1.0)  # lhsT for K=1 broadcast matmul
        # attn2: pair-packed normalized attention [128(e of 2 heads), 2, s]
        attn2 = persist.tile([P, 2, s], bf16, name="attn2")

        # ---- helpers (PE work runs in the fill PSUM pool) -------------
        def qk_proj(xget, w_sb, b_sb, dst, p, sh):
            # dst[:, p, sh-block] = (x @ W_pair.T)^T + bias  for 512 cols
            ps = fillp.tile([P, WSC], f32, tag="fill", name="qkps")
            for k in range(KD):
                nc.tensor.matmul(
                    ps,
                    lhsT=w_sb[:, k, p * P:(p + 1) * P],
                    rhs=xget(k, sh),
                    start=(k == 0),
                    stop=(k == KD - 1),
                )
            nc.vector.tensor_scalar(
                dst[:, p, sh * WSC:(sh + 1) * WSC], ps, b_sb[:, p:p + 1],
                None, ALU.add,
            )

        def v_proj(t):
            ps = fillp.tile([P, WSC], f32, tag="fill", name="vps")
            xvb = xv_blocks[t // 4]
            toff = (t % 4) * P
            for k in range(KD):
                nc.tensor.matmul(
                    ps[:, 0:256],
                    lhsT=xvb[:, k, toff:toff + P],
                    rhs=wv_sb[:, k, :],
                    start=(k == 0),
                    stop=(k == KD - 1),
                )
            for h in range(NH):
                nc.vector.tensor_tensor(
                    v_aug[:, h, t, 0:64],
                    ps[:, h * 64:(h + 1) * 64],
                    bv_sb[:, h * 64:(h + 1) * 64],
                    ALU.add,
                )

        def out_chunk_stage(ci, nh_i, cell):
            # half an out chunk: one nh column-half (2 mms + cast)
            if nh_i == 0:
                cell.append(ospool.tile([P, D], bf16, tag="ob", name="ob"))
            ob = cell[0]
            op = fillp.tile([P, WSC], f32, tag="fill", name="op")
            for p in range(2):
                nc.tensor.matmul(
                    op,
                    lhsT=attn2[:, p, ci * P:(ci + 1) * P],
                    rhs=wo_sb[:, p, nh_i * 512:(nh_i + 1) * 512],
                    start=(p == 0),
                    stop=(p == 1),
                )
            nc.vector.tensor_copy(ob[:, nh_i * 512:(nh_i + 1) * 512], op)
            if nh_i == 1:
                nc.sync.dma_start(out=out[ci * P:(ci + 1) * P, :], in_=ob)

        def out_chunk(ci):
            # out rows [ci*128, (ci+1)*128) ; contract attn2 over both pairs
            cell = []
            out_chunk_stage(ci, 0, cell)
            out_chunk_stage(ci, 1, cell)

        def normalize(p, sh, ch0, ch1):
            soff = sh * WSC
            # drain chains to SBUF first: frees both chain banks after two
            # quick DVE copies so the next round's attn never waits long
            araw = rzpool.tile([P, 2, WSC], f32, tag="araw", name="araw")
            nc.vector.tensor_copy(araw[0:65, 0, :], ch0[0:65, :])
            nc.vector.tensor_copy(araw[0:65, 1, :], ch1[0:65, :])
            # Z (row 64): partition 64 -> 0 shift via SBUF DMA, then
            # broadcast (partition_broadcast needs a partition-0 src)
            z0 = rzpool.tile([1, 2, WSC], f32, tag="z0", name="z0")
            nc.sync.dma_start(out=z0, in_=araw[64:65])
            rz = rzpool.tile([64, 2, WSC], f32, tag="rz", name="rz")
            nc.gpsimd.partition_broadcast(rz, z0)
            nc.vector.reciprocal_approx_fast(rz, rz)
            # even head of pair -> attn2 rows 0:64 directly
            nc.vector.tensor_tensor(
                attn2[0:64, p, soff:soff + WSC],
                araw[0:64, 0, :],
                rz[:, 0, :],
                ALU.mult,
            )
            # odd head: scale to tmp then DMA-shift to rows 64:128
            atmp = rzpool.tile([HD, WSC], bf16, tag="atmp", name="atmp")
            nc.vector.tensor_tensor(atmp, araw[0:64, 1, :], rz[:, 1, :], ALU.mult)
            nc.sync.dma_start(
                out=attn2[64:128, p, soff:soff + WSC], in_=atmp
            )

        # ---- lead-in: k-proj (xk-block paced, sh-major), q(sh0) -------
        def xk_get(k, sh):
            return xk_blocks[sh][:, k, :]

        for sh in range(NSH):
            for p in range(2):
                qk_proj(xk_get, wk_sb, bk_sb, k2, p, sh)
        for p in range(2):
            qk_proj(xq_get, wq_sb, bq_sb, q2, p, 0)

        # ---- filler schedule ------------------------------------------
        # round r = sh*2 + p ; out-proj for sh needs rounds sh*2, sh*2+1
        # normalized, so its 4 chunks spread over rounds sh*2+2, sh*2+3.
        fillers = {r: {} for r in range(2 * NSH)}

        def add_filler(r, sl, job):
            fillers[r].setdefault(sl, []).append(job)

        # v-proj: round-0 filler; xv lands before round 0 starts, so pack
        # two per early slot (v(t) must land before attn(t) at slot t+2)
        for t in range(NT):
            add_filler(0, 2 + (t * 13) // 16, lambda t=t: v_proj(t))
        # q-proj fillers split into two 4-mm half-chains at consecutive
        # slots (same PSUM tile; no other fill allocation may intervene)
        def q_half(p, sh, half, cell):
            if half == 0:
                cell.append(fillp.tile([P, WSC], f32, tag="fill", name="qkps"))
            ps = cell[0]
            for k in range(4 * half, 4 * half + 4):
                nc.tensor.matmul(
                    ps,
                    lhsT=wq_sb[:, k, p * P:(p + 1) * P],
                    rhs=xq_get(k, sh),
                    start=(k == 0),
                    stop=(k == KD - 1),
                )
            if half == 1:
                nc.vector.tensor_scalar(
                    q2[:, p, sh * WSC:(sh + 1) * WSC], ps, bq_sb[:, p:p + 1],
                    None, ALU.add,
                )

        qjobs = [(sh, p) for sh in range(1, NSH) for p in range(2)]
        qslots = [(1, 0), (1, 8), (2, 0), (3, 0), (4, 0), (4, 8)]
        for (r, sl), (sh, p) in zip(qslots, qjobs):
            cell = []
            add_filler(r, sl, lambda sh=sh, p=p, c=cell: q_half(p, sh, 0, c))
            add_filler(r, sl + 1, lambda sh=sh, p=p, c=cell: q_half(p, sh, 1, c))
        # NOTE: out_chunk(sh) depends on normalize(sh*2+1), which is
        # emitted at slot 1 of round sh*2+2 - chunks there must sit at
        # slot >= 2 or the RAW dependency is never formed (stale read).
        # Each chunk is split into two per-nh stages at consecutive slots.
        oslots = {0: [(2, 4), (2, 12), (3, 4), (3, 12)],
                  1: [(4, 4), (4, 12), (5, 2), (5, 8)],
                  2: [(6, 2), (6, 9), (7, 2), (7, 8)]}
        for sh, slots in oslots.items():
            for j, (r, sl) in enumerate(slots):
                ci = sh * 4 + j
                cell = []
                add_filler(r, sl, lambda ci=ci, c=cell: out_chunk_stage(ci, 0, c))
                add_filler(r, sl + 1, lambda ci=ci, c=cell: out_chunk_stage(ci, 1, c))

        # ---- attention rounds -----------------------------------------
        # the previous round's attn-drain + normalize are emitted in the
        # first slots of the next round, so the score/exp stream never
        # waits behind them at a boundary
        pending = []
        for r in range(2 * NSH):
            sh, p = r // 2, r % 2
            soff = sh * WSC
            heads = (2 * p, 2 * p + 1)
            lag = 2
            ch0 = chp.tile([P, WSC], f32, tag="ch", name="ch0")
            ch1 = chp.tile([P, WSC], f32, tag="ch", name="ch1")
            ets = {}

            def attn_step(t, ch0=ch0, ch1=ch1, heads=heads, ets=ets):
                et = ets.pop(t)
                nc.tensor.matmul(
                    ch0[0:65, :],
                    lhsT=v_aug[:, heads[0], t, :],
                    rhs=et[:, 0:WSC],
                    start=(t == 0),
                    stop=(t == NT - 1),
                )
                nc.tensor.matmul(
                    ch1[0:65, :],
                    lhsT=v_aug[:, heads[1], t, :],
                    rhs=et[:, WSC:2 * WSC],
                    start=(t == 0),
                    stop=(t == NT - 1),
                )

            for t in range(NT):
                sc = scp.tile([P, 2 * WSC], f32, tag="sc", name="sc")
                for hi in range(2):
                    rlo, rhi = (0, 64) if hi == 0 else (64, 128)
                    nc.tensor.matmul(
                        sc[:, hi * WSC:(hi + 1) * WSC],
                        lhsT=k2[rlo:rhi, p, t * P:(t + 1) * P],
                        rhs=q2[rlo:rhi, p, soff:soff + WSC],
                        start=True,
                        stop=True,
                        tile_position=(rlo, 0),
                    )
                et = epool.tile([P, 2 * WSC], bf16, tag="exp", name="et")
                nc.scalar.activation(et, sc, AF.Exp, bias=0.0, scale=0.125)
                ets[t] = et
                if t == 0:  # drain previous round's chains
                    for job in pending[:-1]:
                        job()
                elif t == 1 and pending:
                    pending[-1]()  # previous round's normalize
                for job in fillers[r].get(t, []):
                    job()
                if t >= lag:
                    attn_step(t - lag)
            pending = [
                lambda t=t, f=attn_step: f(t) for t in range(NT - lag, NT)
            ]
            if r < 2 * NSH - 1:
                pending.append(
                    lambda p=p, sh=sh, a=ch0, b=ch1: normalize(p, sh, a, b)
                )
            last = (p, sh, ch0, ch1)

        # ---- tail: drain last round; sliced normalize + out-proj ------
        # (PE K=1 ones-matmul broadcast instead of the slow gpsimd
        # dispatch, 256-col slices so out-proj/DMA pipeline per slice)
        for job in pending:
            job()
        p, sh, ch0, ch1 = last
        soff = sh * WSC
        araw = rzpool.tile([P, 2, WSC], f32, tag="araw", name="araw_t")
        nc.vector.tensor_copy(araw[0:65, 0, :], ch0[0:65, :])
        nc.vector.tensor_copy(araw[0:65, 1, :], ch1[0:65, :])
        z0 = rzpool.tile([1, 2, WSC], f32, tag="z0", name="z0_t")
        nc.sync.dma_start(out=z0, in_=araw[64:65])
        zr = rzpool.tile([1, 2, WSC], f32, tag="zr", name="zr_t")
        nc.vector.reciprocal_approx_fast(zr, z0)
        HW_ = WSC // 2
        for sl in range(2):
            cs = slice(sl * HW_, (sl + 1) * HW_)
            ocs = slice(soff + sl * HW_, soff + (sl + 1) * HW_)
            rzp = fillp.tile([P, WSC], f32, tag="fill", name="rzp")
            nc.tensor.matmul(rzp[0:64, 0:HW_], lhsT=ones_sb,
                             rhs=zr[0:1, 0, cs], start=True, stop=True)
            nc.tensor.matmul(rzp[0:64, HW_:2 * HW_], lhsT=ones_sb,
                             rhs=zr[0:1, 1, cs], start=True, stop=True)
            nc.vector.tensor_tensor(
                attn2[0:64, p, ocs], araw[0:64, 0, cs],
                rzp[0:64, 0:HW_], ALU.mult,
            )
            atmp = rzpool.tile([HD, HW_], bf16, tag="atmp", name="atmp_t")
            nc.vector.tensor_tensor(
                atmp, araw[0:64, 1, cs], rzp[0:64, HW_:2 * HW_], ALU.mult,
            )
            nc.sync.dma_start(out=attn2[64:128, p, ocs], in_=atmp)
            out_chunk((NSH - 1) * 4 + 2 * sl)
            out_chunk((NSH - 1) * 4 + 2 * sl + 1)

    nc.compile()
    return nc


def get_bass(s=S):
    if s not in _BUILD_CACHE:
        _BUILD_CACHE[s] = build_bass(s)
    return _BUILD_CACHE[s]


def make_in_maps(query, key, value, Wq, bq, Wk, bk, Wv, bv, Wo):
    """Host-side sharding: per-core input dict for core = b*4 + g."""
    in_maps = []
    for core in range(8):
        b, g = core // 4, core % 4
        cs = slice(g * 256, (g + 1) * 256)
        # pair-packed: wo_h[hd + 64*(h%2), h//2, :] = Wo[:, g*256 + h*64 + hd]
        wo_h = (
            np.ascontiguousarray(Wo[:, cs].T)  # [256(h*64+hd), 1024]
            .reshape(2, P, D)
            .transpose(1, 0, 2)
        )
        m = {
            "xq_t": np.ascontiguousarray(query[:, b, :].T).astype(BF16),
            "xk_t": np.ascontiguousarray(key[:, b, :].T).astype(BF16),
            "xv_t": np.ascontiguousarray(value[:, b, :].T).astype(BF16),
            "wq_t": np.ascontiguousarray(Wq[cs, :].T).astype(BF16),
            "wk_t": np.ascontiguousarray(Wk[cs, :].T).astype(BF16),
            "wv_t": np.ascontiguousarray(Wv[cs, :].T).astype(BF16),
            "wo_h": np.ascontiguousarray(wo_h).astype(BF16),
            "bq2": np.ascontiguousarray(bq[cs].reshape(2, P).T).astype(np.float32),
            "bk2": np.ascontiguousarray(bk[cs].reshape(2, P).T).astype(np.float32),
            "bv4": np.ascontiguousarray(
                np.broadcast_to(bv[cs], (P, 256))
            ).astype(np.float32),
        }
        in_maps.append(m)
    return in_maps


def kernel(query, key, value, Wq, bq, Wk, bk, Wv, bv, Wo, bo):
    from concourse.bass_utils import run_bass_kernel_spmd

    query = np.asarray(query, dtype=np.float32)
    key = np.asarray(key, dtype=np.float32)
    value = np.asarray(value, dtype=np.float32)
    Wq = np.asarray(Wq, dtype=np.float32)
    Wk = np.asarray(Wk, dtype=np.float32)
    Wv = np.asarray(Wv, dtype=np.float32)
    Wo = np.asarray(Wo, dtype=np.float32)

    nc = get_bass(S)
    in_maps = make_in_maps(query, key, value, Wq, bq, Wk, bk, Wv, bv, Wo)
    res = run_bass_kernel_spmd(nc, in_maps, core_ids=list(range(8)))
    outs = [res.results[c]["out"] for c in range(8)]

    full = np.empty((S, B, D), dtype=np.float32)
    bo32 = np.asarray(bo, dtype=np.float32)
    for b in range(B):
        acc = outs[b * 4].astype(np.float32).copy()
        for g in range(1, 4):
            acc += outs[b * 4 + g]
        full[:, b, :] = acc + bo32[None, :]
    return full


# revision 51
# speedup vs baseline: 1.0985x; 1.0040x over previous
"""Multi-head attention (S=2048, B=2, D=1024, H=16, Hd=64) on 8 trn2 cores.

Sharding: core = (batch b, head-group g of 4 heads)  -> 2*4 = 8 cores.
Each core computes the full attention for its 4 heads / 1 batch and a
partial output projection (row-parallel Wo); the host sums the 4 partials
per batch and adds bo.

Schedule (v3): software-pipelined around the ACT engine's exp wall.
  - 8 attention rounds of (sh in 4 s-blocks of 512, p in 2 head-pairs);
    per t-step the PE does 2 score mms (row-paired heads at tile_position
    0/64) + 2 attn chain mms (emitted with lag 2 so chain-buffer reuse
    stalls never block the score stream); ACT does one exp over
    [128, 1024] (both heads packed side by side in one PSUM score tile).
  - PSUM: scores 2x[128,1024] (4 banks) + chains 2x[128,512] (2 banks)
    + fill pool 2x[128,512] (2 banks) for proj/out-proj work that is
    interleaved into the rounds as PE filler (keeps the PE p-state up).
  - DMA order: wk, xk, wq, xq[sh0], wv, xv, xq[sh1..3], wo - so the
    k-projection starts as soon as the first xk tile lands and round 0
    starts right after q2[sh0]; the v-projection runs as round-0 filler
    (attn lag 4 there so it never blocks the score stream).
  - normalize: chains are drained to SBUF immediately (frees the chain
    PSUM bank for the next round after one DVE copy); Z goes partition
    64 -> 0 via a small gpsimd-issued SBUF DMA, then gpsimd
    partition_broadcast (which only honors partition-0 sources), DVE
    reciprocal and the scaling multiplies - all off the PE critical path.
  - out-proj is chunked per 128 output rows and interleaved as filler;
    each chunk DMAs out immediately from the SP queue.
"""

import sys

for _p in ("/opt/trn_rl_repo", "/root/.axon_site/_ro/trn_rl_repo"):
    if _p not in sys.path:
        sys.path.insert(0, _p)

import numpy as np
import ml_dtypes

S = 2048
B = 2
D = 1024
H = 16
HD = 64
NH = 4  # heads per core
P = 128
KD = D // P  # 8 contraction tiles for projections
NT = S // P  # 16 t tiles
WSC = 512  # s-columns per round
NSH = S // WSC  # 4 s-blocks

BF16 = ml_dtypes.bfloat16

_BUILD_CACHE = {}


def build_bass(s=S):
    """Build the per-core Bass module (same program for all 8 cores)."""
    import concourse.bacc as bacc
    import concourse.bass as bass
    import concourse.mybir as mybir
    import concourse.tile as tile

    f32 = mybir.dt.float32
    f32r = mybir.dt.float32r
    bf16 = mybir.dt.bfloat16
    AF = mybir.ActivationFunctionType
    ALU = mybir.AluOpType

    nc = bacc.Bacc("TRN2", target_bir_lowering=False, debug=False, num_devices=8)

    # x/w tensors are pre-rearranged on the host so every DMA is one
    # contiguous multi-KB run per partition (descriptor-lean)
    NB = s // WSC
    xq = nc.dram_tensor("xq_t", [P, NB, KD, WSC], bf16, kind="ExternalInput").ap()
    xk = nc.dram_tensor("xk_t", [P, NB, KD, WSC], bf16, kind="ExternalInput").ap()
    xv = nc.dram_tensor("xv_t", [P, NB, KD, WSC], bf16, kind="ExternalInput").ap()
    wq = nc.dram_tensor("wq_t", [P, KD, 256], bf16, kind="ExternalInput").ap()
    wk = nc.dram_tensor("wk_t", [P, KD, 256], bf16, kind="ExternalInput").ap()
    wv = nc.dram_tensor("wv_t", [P, KD, 256], bf16, kind="ExternalInput").ap()
    wo = nc.dram_tensor("wo_h", [P, 2, D], bf16, kind="ExternalInput").ap()
    bq2 = nc.dram_tensor("bq2", [P, 2], f32, kind="ExternalInput").ap()
    bk2 = nc.dram_tensor("bk2", [P, 2], f32, kind="ExternalInput").ap()
    bv4 = nc.dram_tensor("bv4", [P, 256], f32, kind="ExternalInput").ap()
    out = nc.dram_tensor("out", [s, D], bf16, kind="ExternalOutput").ap()

    from contextlib import ExitStack

    with tile.TileContext(nc) as tc, ExitStack() as ctx:
        consts = ctx.enter_context(tc.tile_pool(name="consts", bufs=1))
        persist = ctx.enter_context(tc.tile_pool(name="persist", bufs=1))
        xkpool = ctx.enter_context(tc.tile_pool(name="xkpool", bufs=NSH))
        xvpool = ctx.enter_context(tc.tile_pool(name="xvpool", bufs=NSH))
        xqpool = ctx.enter_context(tc.tile_pool(name="xqpool", bufs=1))
        epool = ctx.enter_context(tc.tile_pool(name="epool", bufs=8))
        rzpool = ctx.enter_context(tc.tile_pool(name="rzpool", bufs=2))
        ospool = ctx.enter_context(tc.tile_pool(name="ospool", bufs=3))
        scp = ctx.enter_context(tc.tile_pool(name="scp", bufs=2, space="PSUM"))
        chp = ctx.enter_context(tc.tile_pool(name="chp", bufs=2, space="PSUM"))
        fillp = ctx.enter_context(tc.tile_pool(name="fillp", bufs=2, space="PSUM"))

        # ---- DMA order: wk, xk, wq, xq[sh0], wv, xv, xq[sh1..], wo ----
        wk_sb = consts.tile([P, KD, 256], bf16, name="wk_sb")
        nc.sync.dma_start(out=wk_sb, in_=wk)
        bk_sb = consts.tile([P, 2], f32, name="bk_sb")
        nc.sync.dma_start(out=bk_sb, in_=bk2)

        # few, large DMAs: each dma_start costs ~0.7us of serial SP issue
        # time. xk lands in four 512-column blocks so the first k-proj
        # chain (which contracts all k but only needs 512 s-columns)
        # starts as soon as block 0 arrives.
        xk_blocks = []
        for sh in range(NSH):
            t_ = xkpool.tile([P, KD, WSC], bf16, tag="xk", name=f"xk{sh}")
            nc.sync.dma_start(out=t_, in_=xk[:, sh])
            xk_blocks.append(t_)

        wq_sb = consts.tile([P, KD, 256], bf16, name="wq_sb")
        nc.sync.dma_start(out=wq_sb, in_=wq)
        bq_sb = consts.tile([P, 2], f32, name="bq_sb")
        nc.sync.dma_start(out=bq_sb, in_=bq2)

        xq0_tile = xqpool.tile([P, KD, WSC], bf16, tag="xq0", name="xq0")
        nc.sync.dma_start(out=xq0_tile, in_=xq[:, 0])

        wv_sb = consts.tile([P, KD, 256], bf16, name="wv_sb")
        nc.sync.dma_start(out=wv_sb, in_=wv)
        bv_sb = consts.tile([P, 256], f32, name="bv_sb")
        nc.sync.dma_start(out=bv_sb, in_=bv4)

        # xv in four t-column blocks: v_proj(t) only waits for its block
        xv_blocks = []
        for b in range(NSH):
            t_ = xvpool.tile([P, KD, WSC], bf16, tag="xv", name=f"xv{b}")
            nc.sync.dma_start(out=t_, in_=xv[:, b])
            xv_blocks.append(t_)

        xq1_tile = xqpool.tile([P, NSH - 1, KD, WSC], bf16, tag="xq1", name="xq1")
        nc.sync.dma_start(out=xq1_tile, in_=xq[:, 1:])

        def xq_get(k, sh):
            return xq0_tile[:, k, :] if sh == 0 else xq1_tile[:, sh - 1, k, :]

        wo_sb = consts.tile([P, 2, D], bf16, name="wo_sb")
        nc.sync.dma_start(out=wo_sb, in_=wo)

        # ---- persistent activations -----------------------------------
        q2 = persist.tile([P, 2, s], bf16, name="q2")
        k2 = persist.tile([P, 2, s], bf16, name="k2")
        v_aug = persist.tile([P, NH, NT, 65], bf16, name="v_aug")
        nc.vector.memset(v_aug, 1.0)  # col 64 stays 1.0 = Z ones column
        ones_sb = consts.tile([1, 64], f32, name="ones_sb")
        nc.vector.memset(ones_sb, 1.0)  # lhsT for K=1 broadcast matmul
        # attn2: pair-packed normalized attention [128(e of 2 heads), 2, s]
        attn2 = persist.tile([P, 2, s], bf16, name="attn2")

        # ---- helpers (PE work runs in the fill PSUM pool) -------------
        def qk_proj(xget, w_sb, b_sb, dst, p, sh):
            # dst[:, p, sh-block] = (x @ W_pair.T)^T + bias  for 512 cols
            ps = fillp.tile([P, WSC], f32, tag="fill", name="qkps")
            for k in range(KD):
                nc.tensor.matmul(
                    ps,
                    lhsT=w_sb[:, k, p * P:(p + 1) * P],
                    rhs=xget(k, sh),
                    start=(k == 0),
                    stop=(k == KD - 1),
                )
            nc.vector.tensor_scalar(
                dst[:, p, sh * WSC:(sh + 1) * WSC], ps, b_sb[:, p:p + 1],
                None, ALU.add,
            )

        def v_proj(t):
            ps = fillp.tile([P, WSC], f32, tag="fill", name="vps")
            xvb = xv_blocks[t // 4]
            toff = (t % 4) * P
            for k in range(KD):
                nc.tensor.matmul(
                    ps[:, 0:256],
                    lhsT=xvb[:, k, toff:toff + P],
                    rhs=wv_sb[:, k, :],
                    start=(k == 0),
                    stop=(k == KD - 1),
                )
            for h in range(NH):
                nc.vector.tensor_tensor(
                    v_aug[:, h, t, 0:64],
                    ps[:, h * 64:(h + 1) * 64],
                    bv_sb[:, h * 64:(h + 1) * 64],
                    ALU.add,
                )

        def out_chunk_stage(ci, nh_i, cell):
            # half an out chunk: one nh column-half (2 mms + cast)
            if nh_i == 0:
                cell.append(ospool.tile([P, D], bf16, tag="ob", name="ob"))
            ob = cell[0]
            op = fillp.tile([P, WSC], f32, tag="fill", name="op")
            for p in range(2):
                nc.tensor.matmul(
                    op,
                    lhsT=attn2[:, p, ci * P:(ci + 1) * P],
                    rhs=wo_sb[:, p, nh_i * 512:(nh_i + 1) * 512],
                    start=(p == 0),
                    stop=(p == 1),
                )
            nc.vector.tensor_copy(ob[:, nh_i * 512:(nh_i + 1) * 512], op)
            if nh_i == 1:
                nc.sync.dma_start(out=out[ci * P:(ci + 1) * P, :], in_=ob)

        def out_chunk(ci):
            # out rows [ci*128, (ci+1)*128) ; contract attn2 over both pairs
            cell = []
            out_chunk_stage(ci, 0, cell)
            out_chunk_stage(ci, 1, cell)

        def normalize(p, sh, ch0, ch1):
            soff = sh * WSC
            # drain chains to SBUF first: frees both chain banks after two
            # quick DVE copies so the next round's attn never waits long
            araw = rzpool.tile([P, 2, WSC], f32, tag="araw", name="araw")
            nc.vector.tensor_copy(araw[0:65, 0, :], ch0[0:65, :])
            nc.vector.tensor_copy(araw[0:65, 1, :], ch1[0:65, :])
            # Z (row 64): partition 64 -> 0 shift via SBUF DMA, then
            # broadcast (partition_broadcast needs a partition-0 src)
            z0 = rzpool.tile([1, 2, WSC], f32, tag="z0", name="z0")
            nc.sync.dma_start(out=z0, in_=araw[64:65])
            rz = rzpool.tile([64, 2, WSC], f32, tag="rz", name="rz")
            nc.gpsimd.partition_broadcast(rz, z0)
            nc.vector.reciprocal_approx_fast(rz, rz)
            # even head of pair -> attn2 rows 0:64 directly
            nc.vector.tensor_tensor(
                attn2[0:64, p, soff:soff + WSC],
                araw[0:64, 0, :],
                rz[:, 0, :],
                ALU.mult,
            )
            # odd head: scale to tmp then DMA-shift to rows 64:128
            atmp = rzpool.tile([HD, WSC], bf16, tag="atmp", name="atmp")
            nc.vector.tensor_tensor(atmp, araw[0:64, 1, :], rz[:, 1, :], ALU.mult)
            nc.sync.dma_start(
                out=attn2[64:128, p, soff:soff + WSC], in_=atmp
            )

        # ---- lead-in: k-proj (xk-block paced, sh-major), q(sh0) -------
        def xk_get(k, sh):
            return xk_blocks[sh][:, k, :]

        for sh in range(NSH):
            for p in range(2):
                qk_proj(xk_get, wk_sb, bk_sb, k2, p, sh)
        for p in range(2):
            qk_proj(xq_get, wq_sb, bq_sb, q2, p, 0)

        # ---- filler schedule ------------------------------------------
        # round r = sh*2 + p ; out-proj for sh needs rounds sh*2, sh*2+1
        # normalized, so its 4 chunks spread over rounds sh*2+2, sh*2+3.
        fillers = {r: {} for r in range(2 * NSH)}

        def add_filler(r, sl, job):
            fillers[r].setdefault(sl, []).append(job)

        # v-proj: round-0 filler; xv lands before round 0 starts, so pack
        # two per early slot (v(t) must land before attn(t) at slot t+2)
        for t in range(NT):
            add_filler(0, 2 + (t * 13) // 16, lambda t=t: v_proj(t))
        # q-proj fillers split into two 4-mm half-chains at consecutive
        # slots (same PSUM tile; no other fill allocation may intervene)
        def q_half(p, sh, half, cell):
            if half == 0:
                cell.append(fillp.tile([P, WSC], f32, tag="fill", name="qkps"))
            ps = cell[0]
            for k in range(4 * half, 4 * half + 4):
                nc.tensor.matmul(
                    ps,
                    lhsT=wq_sb[:, k, p * P:(p + 1) * P],
                    rhs=xq_get(k, sh),
                    start=(k == 0),
                    stop=(k == KD - 1),
                )
            if half == 1:
                nc.vector.tensor_scalar(
                    q2[:, p, sh * WSC:(sh + 1) * WSC], ps, bq_sb[:, p:p + 1],
                    None, ALU.add,
                )

        qjobs = [(sh, p) for sh in range(1, NSH) for p in range(2)]
        qslots = [(1, 0), (1, 8), (2, 0), (3, 0), (4, 0), (4, 8)]
        for (r, sl), (sh, p) in zip(qslots, qjobs):
            cell = []
            add_filler(r, sl, lambda sh=sh, p=p, c=cell: q_half(p, sh, 0, c))
            add_filler(r, sl + 1, lambda sh=sh, p=p, c=cell: q_half(p, sh, 1, c))
        # NOTE: out_chunk(sh) depends on normalize(sh*2+1), which is
        # emitted at slot 1 of round sh*2+2 - chunks there must sit at
        # slot >= 2 or the RAW dependency is never formed (stale read).
        # Each chunk is split into two per-nh stages at consecutive slots.
        oslots = {0: [(2, 4), (2, 12), (3, 4), (3, 12)],
                  1: [(4, 4), (4, 12), (5, 2), (5, 8)],
                  2: [(6, 2), (6, 9), (7, 2), (7, 8)]}
        for sh, slots in oslots.items():
            for j, (r, sl) in enumerate(slots):
                ci = sh * 4 + j
                cell = []
                add_filler(r, sl, lambda ci=ci, c=cell: out_chunk_stage(ci, 0, c))
                add_filler(r, sl + 1, lambda ci=ci, c=cell: out_chunk_stage(ci, 1, c))

        # ---- attention rounds -----------------------------------------
        # the previous round's attn-drain + normalize are emitted in the
        # first slots of the next round, so the score/exp stream never
        # waits behind them at a boundary
        pending = []
        for r in range(2 * NSH):
            sh, p = r // 2, r % 2
            soff = sh * WSC
            heads = (2 * p, 2 * p + 1)
            lag = 2
            ch0 = chp.tile([P, WSC], f32, tag="ch", name="ch0")
            ch1 = chp.tile([P, WSC], f32, tag="ch", name="ch1")
            ets = {}

            def attn_step(t, ch0=ch0, ch1=ch1, heads=heads, ets=ets):
                et = ets.pop(t)
                nc.tensor.matmul(
                    ch0[0:65, :],
                    lhsT=v_aug[:, heads[0], t, :],
                    rhs=et[:, 0:WSC],
                    start=(t == 0),
                    stop=(t == NT - 1),
                )
                nc.tensor.matmul(
                    ch1[0:65, :],
                    lhsT=v_aug[:, heads[1], t, :],
                    rhs=et[:, WSC:2 * WSC],
                    start=(t == 0),
                    stop=(t == NT - 1),
                )

            for t in range(NT):
                sc = scp.tile([P, 2 * WSC], f32, tag="sc", name="sc")
                for hi in range(2):
                    rlo, rhi = (0, 64) if hi == 0 else (64, 128)
                    nc.tensor.matmul(
                        sc[:, hi * WSC:(hi + 1) * WSC],
                        lhsT=k2[rlo:rhi, p, t * P:(t + 1) * P],
                        rhs=q2[rlo:rhi, p, soff:soff + WSC],
                        start=True,
                        stop=True,
                        tile_position=(rlo, 0),
                    )
                et = epool.tile([P, 2 * WSC], bf16, tag="exp", name="et")
                nc.scalar.activation(et, sc, AF.Exp, bias=0.0, scale=0.125)
                ets[t] = et
                if t == 0:  # drain previous round's chains
                    for job in pending[:-1]:
                        job()
                elif t == 1 and pending:
                    pending[-1]()  # previous round's normalize
                for job in fillers[r].get(t, []):
                    job()
                if t >= lag:
                    attn_step(t - lag)
            pending = [
                lambda t=t, f=attn_step: f(t) for t in range(NT - lag, NT)
            ]
            if r < 2 * NSH - 1:
                pending.append(
                    lambda p=p, sh=sh, a=ch0, b=ch1: normalize(p, sh, a, b)
                )
            last = (p, sh, ch0, ch1)

        # ---- tail: drain last round; sliced normalize + out-proj ------
        # (PE K=1 ones-matmul broadcast instead of the slow gpsimd
        # dispatch, 256-col slices so out-proj/DMA pipeline per slice)
        for job in pending:
            job()
        p, sh, ch0, ch1 = last
        soff = sh * WSC
        araw = rzpool.tile([P, 2, WSC], f32, tag="araw", name="araw_t")
        nc.vector.tensor_copy(araw[0:65, 0, :], ch0[0:65, :])
        nc.vector.tensor_copy(araw[0:65, 1, :], ch1[0:65, :])
        z0 = rzpool.tile([1, 2, WSC], f32, tag="z0", name="z0_t")
        nc.sync.dma_start(out=z0, in_=araw[64:65])
        zr = rzpool.tile([1, 2, WSC], f32, tag="zr", name="zr_t")
        nc.vector.reciprocal_approx_fast(zr, z0)
        HW_ = WSC // 2
        for sl in range(2):
            cs = slice(sl * HW_, (sl + 1) * HW_)
            ocs = slice(soff + sl * HW_, soff + (sl + 1) * HW_)
            rzp = fillp.tile([P, WSC], f32, tag="fill", name="rzp")
            nc.tensor.matmul(rzp[0:64, 0:HW_], lhsT=ones_sb,
                             rhs=zr[0:1, 0, cs], start=True, stop=True)
            nc.tensor.matmul(rzp[0:64, HW_:2 * HW_], lhsT=ones_sb,
                             rhs=zr[0:1, 1, cs], start=True, stop=True)
            nc.vector.tensor_tensor(
                attn2[0:64, p, ocs], araw[0:64, 0, cs],
                rzp[0:64, 0:HW_], ALU.mult,
            )
            atmp = rzpool.tile([HD, HW_], bf16, tag="atmp", name="atmp_t")
            nc.vector.tensor_tensor(
                atmp, araw[0:64, 1, cs], rzp[0:64, HW_:2 * HW_], ALU.mult,
            )
            nc.sync.dma_start(out=attn2[64:128, p, ocs], in_=atmp)
            out_chunk((NSH - 1) * 4 + 2 * sl)
            out_chunk((NSH - 1) * 4 + 2 * sl + 1)

    nc.compile()
    return nc


def get_bass(s=S):
    if s not in _BUILD_CACHE:
        _BUILD_CACHE[s] = build_bass(s)
    return _BUILD_CACHE[s]


def _x_r(x_b):
    # [S, D] -> [P, NSH, KD, WSC]: partition-major, s-block-major layout
    # so every device DMA is one contiguous run per partition
    xt = x_b.T  # [D, S]
    v = xt.reshape(KD, P, S // WSC, WSC).transpose(1, 2, 0, 3)
    return np.ascontiguousarray(v).astype(BF16)


def _w_r(W_cs):
    # [256, D] -> [P, KD, 256]
    v = W_cs.T.reshape(KD, P, 256).transpose(1, 0, 2)
    return np.ascontiguousarray(v).astype(BF16)


def make_in_maps(query, key, value, Wq, bq, Wk, bk, Wv, bv, Wo):
    """Host-side sharding: per-core input dict for core = b*4 + g."""
    in_maps = []
    for core in range(8):
        b, g = core // 4, core % 4
        cs = slice(g * 256, (g + 1) * 256)
        # pair-packed: wo_h[hd + 64*(h%2), h//2, :] = Wo[:, g*256 + h*64 + hd]
        wo_h = (
            np.ascontiguousarray(Wo[:, cs].T)  # [256(h*64+hd), 1024]
            .reshape(2, P, D)
            .transpose(1, 0, 2)
        )
        m = {
            "xq_t": _x_r(query[:, b, :]),
            "xk_t": _x_r(key[:, b, :]),
            "xv_t": _x_r(value[:, b, :]),
            "wq_t": _w_r(Wq[cs, :]),
            "wk_t": _w_r(Wk[cs, :]),
            "wv_t": _w_r(Wv[cs, :]),
            "wo_h": np.ascontiguousarray(wo_h).astype(BF16),
            "bq2": np.ascontiguousarray(bq[cs].reshape(2, P).T).astype(np.float32),
            "bk2": np.ascontiguousarray(bk[cs].reshape(2, P).T).astype(np.float32),
            "bv4": np.ascontiguousarray(
                np.broadcast_to(bv[cs], (P, 256))
            ).astype(np.float32),
        }
        in_maps.append(m)
    return in_maps


def kernel(query, key, value, Wq, bq, Wk, bk, Wv, bv, Wo, bo):
    from concourse.bass_utils import run_bass_kernel_spmd

    query = np.asarray(query, dtype=np.float32)
    key = np.asarray(key, dtype=np.float32)
    value = np.asarray(value, dtype=np.float32)
    Wq = np.asarray(Wq, dtype=np.float32)
    Wk = np.asarray(Wk, dtype=np.float32)
    Wv = np.asarray(Wv, dtype=np.float32)
    Wo = np.asarray(Wo, dtype=np.float32)

    nc = get_bass(S)
    in_maps = make_in_maps(query, key, value, Wq, bq, Wk, bk, Wv, bv, Wo)
    res = run_bass_kernel_spmd(nc, in_maps, core_ids=list(range(8)))
    outs = [res.results[c]["out"] for c in range(8)]

    full = np.empty((S, B, D), dtype=np.float32)
    bo32 = np.asarray(bo, dtype=np.float32)
    for b in range(B):
        acc = outs[b * 4].astype(np.float32).copy()
        for g in range(1, 4):
            acc += outs[b * 4 + g]
        full[:, b, :] = acc + bo32[None, :]
    return full
